# revision 4
# baseline (speedup 1.0000x reference)
"""GAT layer kernel for Trainium2, 8 NeuronCores — dst-major edge layout.

Math (exact reformulation of the reference):
  proj = x @ W1;  a = proj . attn_src (per head);  d = proj . attn_dst
  score_e = leaky_relu(a[src] + d[dst], 0.2)
  exp(leaky_relu(s)) = max(exp(s), exp(0.2 s))       [monotonicity of exp]
                     = max(ea[src]*ed[dst], ea2[src]*ed2[dst])
  with ea = exp(a), ea2 = exp(0.2 a), ed = exp(d), ed2 = exp(0.2 d).
  (The segment-max subtraction is a softmax no-op and is dropped.)
  U[n]     = sum_{dst=n} expv_e * proj[src_e];  denom[n] = sum expv_e
  y = (elu(U/denom + b1) + 1) @ Wf + (bf - sum(Wf))

Device strategy:
  Nodes are assigned to (core, window, partition) by host permutation with
  windows of 128 dst nodes of similar (degA, degB) so per-partition edge
  lists pad tightly.  Edge slot (p, f): partition p = dst, f along free dim.
  Phase A builds a DRAM node-row table [proj bf16(128) | dup-pairs of
  ea/ea2 bf16(16) | pad] (512B rows) in two tables split at node 31744 so
  gather indices fit int16; row 0 of each is an all-zero sentinel used for
  padding slots (expv = 0).
  Phase B processes windows in pairs (one big-F with one small-F rank)
  sharing a gather region [A1|A2|B1|B2] so gather calls merge across the
  pair: dma_gather rows by src, expv = max(ea*ed, ea2*ed2) with the
  per-window ed table resident in SBUF, g = proj * expv, then per-slot
  identity-weight matmuls accumulate [U | denom] into PSUM (the identity
  stays loaded in the PE array for the whole phase), then normalize, bias,
  elu, and the final 128->1 projection, software-pipelined one group back.
"""

import numpy as np
import ml_dtypes

import concourse.bacc as bacc
import concourse.tile as tile
import concourse.mybir as mybir
import concourse.bass_utils as bass_utils
from concourse.masks import make_identity

# ---------------- problem constants ----------------
N_NODES = 50000
IN_DIM = 128
HEADS = 4
OUT_DIM = 32
HD = HEADS * OUT_DIM  # 128
N_CORES = 8
N_WIN = 49                      # windows per core
WIN = 128                       # dst nodes per window (one per partition)
NPC_PAD = N_WIN * WIN           # 6272 node slots per core
N_WIN_TOT = N_CORES * N_WIN     # 392
NODES_PAD = N_WIN_TOT * WIN     # 50176
SPLIT = 31744                   # node-id split for the two tables (31 chunks)
CHUNK = 1024                    # phase-A nodes per chunk (49 chunks)
ROW = 256                       # table row, bf16 elems (512B)
EA_OFF = 128                    # row cols 128:144 hold dup-pairs ea|ea2
TBLA_ROWS = SPLIT + 2           # sentinel + nodes [0, 31744)
TBLB_ROWS = (NODES_PAD - SPLIT) + 2  # sentinel + nodes [31744, 50176)
GCHUNK = 8                      # gather tiles per call (1024-idx ucode limit)
SCRATCH = 16384                 # swdge ring bytes (default; HW-fixed)

F32 = mybir.dt.float32
BF16 = mybir.dt.bfloat16
I16 = mybir.dt.int16


def _wrap_idx(idx_flat: np.ndarray) -> np.ndarray:
    """[n] int16 -> [128, n//16] wrapped (idx i at [i%16, i//16]), replicated
    down all 128 partitions (8 gpsimd core groups x 16)."""
    n = idx_flat.shape[0]
    assert n % 16 == 0
    w = np.zeros((128, n // 16), dtype=np.int16)
    w[:16, :] = idx_flat.reshape(n // 16, 16).T
    for r in range(1, 8):
        w[16 * r : 16 * (r + 1), :] = w[:16, :]
    return w


def preprocess(edge_index: np.ndarray):
    """Structure-only preprocessing: node permutation, window profiles and
    per-core wrapped gather indices."""
    src0 = edge_index[0].astype(np.int64)
    dst0 = edge_index[1].astype(np.int64)
    loops = np.arange(N_NODES, dtype=np.int64)
    src = np.concatenate([src0, dst0, loops])
    dst = np.concatenate([dst0, src0, loops])

    isB = src >= SPLIT
    degA = np.bincount(dst[~isB], minlength=N_NODES)
    degB = np.bincount(dst[isB], minlength=N_NODES)

    # windows of 128 nodes with similar (degA, degB): tight per-partition pads
    order = np.lexsort((-degB, -degA))          # primary: degA desc
    wFA = np.zeros(N_WIN_TOT, np.int64)
    wFB = np.zeros(N_WIN_TOT, np.int64)
    dA_pad = np.concatenate([degA[order], np.zeros(NODES_PAD - N_NODES, np.int64)])
    dB_pad = np.concatenate([degB[order], np.zeros(NODES_PAD - N_NODES, np.int64)])
    wFA = dA_pad.reshape(N_WIN_TOT, WIN).max(1)
    wFB = dB_pad.reshape(N_WIN_TOT, WIN).max(1)

    # windows sorted by (FA, FB) desc; window j -> core j%8 at rank j//8.
    # Groups of 8 consecutive windows share a rank, so the per-rank common
    # profile (max over the group) should stay tight for FA and FB jointly:
    # FA-sort in blocks of 64, FB-cluster within each block, then a small
    # deterministic swap anneal.
    wo = np.lexsort((-wFB, -wFA))
    out = []
    for b0 in range(0, N_WIN_TOT, 64):
        blk = wo[b0 : b0 + 64]
        out.append(blk[np.argsort(-wFB[blk], kind="stable")])
    worder = np.concatenate(out)

    def _prof_cost(w):
        fa = wFA[w].reshape(N_WIN, 8).max(1)
        fb = wFB[w].reshape(N_WIN, 8).max(1)
        return (fa + fb).sum()

    rng = np.random.default_rng(0)
    c = _prof_cost(worder)
    for _ in range(40000):
        i, j = rng.integers(0, N_WIN_TOT, 2)
        if i // 8 == j // 8:
            continue
        worder[i], worder[j] = worder[j], worder[i]
        c2 = _prof_cost(worder)
        if c2 <= c:
            c = c2
        else:
            worder[i], worder[j] = worder[j], worder[i]
    # common profile per rank: max over the 8 cores sharing the rank
    FA_prof = np.zeros(N_WIN, np.int64)
    FB_prof = np.zeros(N_WIN, np.int64)
    for k in range(N_WIN):
        grp = worder[k * 8 : (k + 1) * 8]
        FA_prof[k] = wFA[grp].max()
        FB_prof[k] = wFB[grp].max()
    F_prof = FA_prof + FB_prof

    # node -> (core, rank k, partition p)
    # global window gw (in sorted-node order) sits at rank k=pos//8, core=pos%8
    gw_rank = np.zeros(N_WIN_TOT, np.int64)
    gw_core = np.zeros(N_WIN_TOT, np.int64)
    for j, gw in enumerate(worder):
        gw_core[gw] = j % 8
        gw_rank[gw] = j // 8

    pos = np.empty(N_NODES, np.int64)           # node -> sorted position
    pos[order] = np.arange(N_NODES)
    node_gw = pos // WIN
    node_p = pos % WIN
    node_core = gw_core[node_gw]
    node_rank = gw_rank[node_gw]

    # map (core, rank, p) -> node for aloc/xmyT and output unpermute
    node_of = np.full((N_CORES, N_WIN, WIN), -1, np.int64)
    node_of[node_core, node_rank, node_p] = np.arange(N_NODES)

    # per-edge slot assignment
    e_core = node_core[dst]
    e_rank = node_rank[dst]
    e_p = node_p[dst]
    e_isB = isB.astype(np.int64)
    # rank within (dst, class): stable sort by (dst, class)
    ek = (pos[dst] << 1) | e_isB
    es = np.argsort(ek, kind="stable")
    ek_s = ek[es]
    new_run = np.ones(len(ek_s), dtype=bool)
    new_run[1:] = ek_s[1:] != ek_s[:-1]
    run_start = np.where(new_run)[0]
    start_per = run_start[np.cumsum(new_run) - 1]
    rank_in = np.arange(len(ek_s)) - start_per
    e_f = np.empty(len(src), np.int64)
    e_f[es] = rank_in

    # ---- rank pairing: groups of 2 ranks (big F with small F) share one
    # gather region laid out [A1 | A2 | B1 | B2], merging gather calls ----
    rsort = np.argsort(-F_prof, kind="stable")
    groups = []
    for i in range(N_WIN // 2):
        groups.append((int(rsort[i]), int(rsort[N_WIN - 1 - i])))
    groups.append((int(rsort[N_WIN // 2]),))  # odd one out

    # per-rank: group base offset contributions
    abase = np.zeros(N_WIN, np.int64)   # f-offset of rank's A segment
    bbase = np.zeros(N_WIN, np.int64)   # f-offset of rank's B segment
    goff = 0
    gspec = []  # per group: (off, [(rank, FA, FB), ...])
    for grp in groups:
        FAg = sum(int(FA_prof[r]) for r in grp)
        ao = goff
        for r in grp:
            abase[r] = ao
            ao += int(FA_prof[r])
        bo = goff + FAg
        for r in grp:
            bbase[r] = bo
            bo += int(FB_prof[r])
        gspec.append((goff, tuple((int(r), int(FA_prof[r]), int(FB_prof[r]))
                                  for r in grp)))
        goff = bo
    F_tot = int(goff)

    e_f = np.where(e_isB == 1, bbase[e_rank] + e_f, abase[e_rank] + e_f)

    # idx value: A: src+1 in tblA ; B: (src-SPLIT)+1 in tblB ; sentinel 0
    e_idx = np.where(e_isB == 1, src - SPLIT + 1, src + 1).astype(np.int16)

    e_slot = e_f * WIN + e_p

    idx_blob = np.zeros((N_CORES, F_tot * WIN), dtype=np.int16)
    idx_blob[e_core, e_slot] = e_idx

    idx_wrapped = np.stack([_wrap_idx(idx_blob[c]) for c in range(N_CORES)])

    return (
        tuple(gspec),
        idx_wrapped,
        node_of,
    )


# ---------------- bass program ----------------
def build_program(gspec, HAS_B1=True):
    # gspec: per group (offset_tiles, ((rank, FA, FB), ...1-2 windows))
    G_MAX = max(sum(fa + fb for _, fa, fb in grp) for _, grp in gspec)
    last_off, last_grp = gspec[-1]
    F_tot = last_off + sum(fa + fb for _, fa, fb in last_grp)

    nc = bacc.Bacc("TRN2", target_bir_lowering=False, debug=False, num_devices=1,
                   num_swdge_queues=1, dynamic_dma_scratch_size=SCRATCH)

    xT_d = nc.dram_tensor("xT", (IN_DIM, NODES_PAD), BF16, kind="ExternalInput").ap()
    xmyT_d = nc.dram_tensor("xmyT", (IN_DIM, NPC_PAD), BF16, kind="ExternalInput").ap()
    w1c_d = nc.dram_tensor("w1c", (IN_DIM, HD + 16), F32, kind="ExternalInput").ap()
    w1d_d = nc.dram_tensor("w1d", (IN_DIM, 16), F32, kind="ExternalInput").ap()
    b1_d = nc.dram_tensor("b1", (1, HD), F32, kind="ExternalInput").ap()
    wf_d = nc.dram_tensor("wf", (1, HD), F32, kind="ExternalInput").ap()
    bfp_d = nc.dram_tensor("bfp", (1, 1), F32, kind="ExternalInput").ap()
    idx_d = nc.dram_tensor("idx", (128, F_tot * 8), I16, kind="ExternalInput").ap()
    y_d = nc.dram_tensor("y", (NPC_PAD, 1), F32, kind="ExternalOutput").ap()
    tblA_d = nc.dram_tensor("tblA", (TBLA_ROWS, ROW), BF16, kind="Internal").ap()
    tblB_d = nc.dram_tensor("tblB", (TBLB_ROWS, ROW), BF16, kind="Internal").ap()

    with tile.TileContext(nc) as tc:
        with tc.tile_pool(name="const", bufs=1) as consts:
            ident = consts.tile([128, 128], BF16)
            make_identity(nc, ident[:])
            w1cf = consts.tile([128, HD + 16], F32)
            nc.sync.dma_start(out=w1cf[:], in_=w1c_d[:, :])
            w1c = consts.tile([128, HD + 16], BF16)
            nc.vector.tensor_copy(out=w1c[:], in_=w1cf[:])
            w1df = consts.tile([128, 16], F32)
            nc.sync.dma_start(out=w1df[:], in_=w1d_d[:, :])
            w1d = consts.tile([128, 16], BF16)
            nc.vector.tensor_copy(out=w1d[:], in_=w1df[:])

            # zero sentinels (row 0 of each table)
            zrow = consts.tile([1, ROW], BF16)
            nc.vector.memset(zrow[:], 0.0)
            nc.sync.dma_start(out=tblA_d[0:1, :], in_=zrow[:])
            nc.sync.dma_start(out=tblB_d[0:1, :], in_=zrow[:])

            # replicated epilogue params via ones-matmul
            ones_row = consts.tile([1, 128], F32)
            nc.vector.memset(ones_row[:], 1.0)
            b1_t = consts.tile([1, HD], F32)
            nc.sync.dma_start(out=b1_t[:], in_=b1_d[:, :])
            wf_t = consts.tile([1, HD], F32)
            nc.sync.dma_start(out=wf_t[:], in_=wf_d[:, :])
            bfp_t = consts.tile([1, 1], F32)
            nc.sync.dma_start(out=bfp_t[:], in_=bfp_d[:, :])
            with tc.tile_pool(name="bc_ps", bufs=1, space="PSUM") as bc_ps:
                bput = bc_ps.tile([128, 2 * HD + 1], F32)
                nc.tensor.matmul(out=bput[:, :HD], lhsT=ones_row[:, :], rhs=b1_t[:, :],
                                 start=True, stop=False)
                nc.tensor.matmul(out=bput[:, HD : 2 * HD], lhsT=ones_row[:, :],
                                 rhs=wf_t[:, :], start=False, stop=False)
                nc.tensor.matmul(out=bput[:, 2 * HD :], lhsT=ones_row[:, :],
                                 rhs=bfp_t[:, :], start=False, stop=True)
                b1_rep = consts.tile([128, HD], F32)
                nc.vector.tensor_copy(out=b1_rep[:], in_=bput[:, :HD])
                wf_rep = consts.tile([128, HD], F32)
                nc.vector.tensor_copy(out=wf_rep[:], in_=bput[:, HD : 2 * HD])
                bfp_rep = consts.tile([128, 1], F32)
                nc.vector.tensor_copy(out=bfp_rep[:], in_=bput[:, 2 * HD :])

            aloc = consts.tile([128, N_WIN, 16], BF16)
            y_all = consts.tile([128, N_WIN], F32)

            # idx blob: prefetch the whole thing into SBUF once
            idx_sb = consts.tile([128, F_tot * 8], I16)
            nc.sync.dma_start(out=idx_sb[:], in_=idx_d[:, :])

            # ---- phase A': per-core dst exp table (SBUF-resident) ----
            with (
                tc.tile_pool(name="pd_x", bufs=2) as pd_x,
                tc.tile_pool(name="pd_ps", bufs=2, space="PSUM") as pd_ps,
            ):
                xmt = pd_x.tile([128, NPC_PAD], BF16, tag="xmt")
                nc.sync.dma_start(out=xmt[:], in_=xmyT_d[:, :])
                for k in range(N_WIN):
                    ps = pd_ps.tile([128, 16], F32, tag="psd")
                    nc.tensor.matmul(
                        out=ps[:], lhsT=xmt[:, k * 128 : (k + 1) * 128],
                        rhs=w1d[:, :], start=True, stop=True,
                    )
                    nc.scalar.activation(aloc[:, k, :], ps[:],
                                         mybir.ActivationFunctionType.Exp)

            # ---- phase A: node tables (chunks of 1024 nodes) ----
            # manual row buffers: pad cols initialized once, full-row stores
            NROW = 5
            rows = [consts.tile([128, 8, ROW], BF16, name=f"rowbuf{i}")
                    for i in range(NROW)]
            for r in rows:
                nc.vector.memset(r[:, :, EA_OFF + 16 :], 0.0)
            with (
                tc.tile_pool(name="pa_x", bufs=6) as pa_x,
                tc.tile_pool(name="pa_ps", bufs=8, space="PSUM") as pa_ps,
            ):
                for c in range(49):
                    xt = pa_x.tile([128, CHUNK], BF16, tag="xt")
                    nc.sync.dma_start(out=xt[:], in_=xT_d[:, c * CHUNK : (c + 1) * CHUNK])
                    row = rows[c % NROW]
                    for i in range(4):  # pairs of 128-node tiles
                        ps = pa_ps.tile([128, 2, HD + 16], F32, tag="ps")
                        for j in range(2):
                            t = 2 * i + j
                            nc.tensor.matmul(
                                out=ps[:, j, :], lhsT=xt[:, t * 128 : (t + 1) * 128],
                                rhs=w1c[:, :], start=True, stop=True,
                            )
                        if i % 2 == 0:
                            nc.vector.tensor_copy(
                                out=row[:, 2 * i : 2 * i + 2, :HD], in_=ps[:, :, :HD])
                        else:
                            nc.scalar.activation(
                                row[:, 2 * i : 2 * i + 2, :HD], ps[:, :, :HD],
                                mybir.ActivationFunctionType.Copy)
                        nc.scalar.activation(row[:, 2 * i : 2 * i + 2,
                                                 EA_OFF : EA_OFF + 16],
                                             ps[:, :, HD : HD + 16],
                                             mybir.ActivationFunctionType.Exp)
                    if c < 31:
                        dst_ap = tblA_d[1 + c * CHUNK : 1 + (c + 1) * CHUNK, :]
                    else:
                        c2 = c - 31
                        dst_ap = tblB_d[1 + c2 * CHUNK : 1 + (c2 + 1) * CHUNK, :]
                    nc.sync.dma_start(
                        out=dst_ap.rearrange("(t p) e -> p t e", p=128),
                        in_=row[:],
                    )

            # ---- phase B ----
            with (
                tc.tile_pool(name="pb_gath", bufs=2) as pb_gath,
                tc.tile_pool(name="pb_g", bufs=2) as pb_g,
                tc.tile_pool(name="pb_m", bufs=3) as pb_m,
                tc.tile_pool(name="pb_s", bufs=3) as pb_s,
                tc.tile_pool(name="pb_ps", bufs=4, space="PSUM") as pb_ps,
            ):
                def emit_epilogue(ps, w):
                    dn = pb_s.tile([128, HEADS], F32, tag="dn")
                    nc.vector.tensor_scalar_max(dn[:], ps[:, HD:], 1e-12)
                    rec = pb_s.tile([128, HEADS, 1], F32, tag="rec")
                    nc.vector.reciprocal(rec[:, :, 0], dn[:])
                    h = pb_s.tile([128, HD], F32, tag="h")
                    nc.vector.tensor_tensor(
                        out=h[:],
                        in0=ps[:, :HD],
                        in1=rec[:, :, :1].to_broadcast([128, HEADS, OUT_DIM]),
                        op=mybir.AluOpType.mult,
                    )
                    if HAS_B1:
                        nc.vector.tensor_tensor(out=h[:], in0=h[:], in1=b1_rep[:],
                                                op=mybir.AluOpType.add)
                    # elu(h)+1 = exp(min(h,0)) + relu(h)  (-1 folded into bfp)
                    hm = pb_s.tile([128, HD], F32, tag="hm")
                    nc.vector.tensor_scalar_min(hm[:], h[:], 0.0)
                    em = pb_s.tile([128, HD], F32, tag="em")
                    nc.scalar.activation(em[:], hm[:],
                                         mybir.ActivationFunctionType.Exp)
                    rl = pb_s.tile([128, HD], F32, tag="rl")
                    nc.scalar.activation(rl[:], h[:],
                                         mybir.ActivationFunctionType.Relu)
                    e = pb_s.tile([128, HD], F32, tag="e")
                    nc.vector.tensor_tensor(out=e[:], in0=em[:], in1=rl[:],
                                            op=mybir.AluOpType.add)
                    scr = pb_s.tile([128, HD], F32, tag="scr")
                    nc.vector.tensor_tensor(out=scr[:], in0=e[:], in1=wf_rep[:],
                                            op=mybir.AluOpType.mult)
                    nc.vector.tensor_reduce(
                        out=y_all[:, w : w + 1], in_=scr[:],
                        axis=mybir.AxisListType.X, op=mybir.AluOpType.add,
                    )

                pending = []  # [(ps, rank), ...] epilogues of the previous group
                for goff, grp in gspec:
                    FAg = sum(fa for _, fa, _ in grp)
                    Fg = FAg + sum(fb for _, _, fb in grp)
                    o8 = goff * 8

                    gath = pb_gath.tile([128, G_MAX, ROW], BF16, tag="gath")
                    for c0 in range(0, FAg, GCHUNK):
                        cn = min(GCHUNK, FAg - c0)
                        nc.gpsimd.dma_gather(
                            out_ap=gath[:, c0 : c0 + cn, :], in_ap=tblA_d[:, :],
                            idxs_ap=idx_sb[:, o8 + c0 * 8 : o8 + (c0 + cn) * 8],
                            num_idxs=cn * 128, num_idxs_reg=cn * 128,
                            elem_size=ROW, queue_num=0,
                        )
                    for c0 in range(FAg, Fg, GCHUNK):
                        cn = min(GCHUNK, Fg - c0)
                        nc.gpsimd.dma_gather(
                            out_ap=gath[:, c0 : c0 + cn, :],
                            in_ap=tblB_d[:, :],
                            idxs_ap=idx_sb[:, o8 + c0 * 8 : o8 + (c0 + cn) * 8],
                            num_idxs=cn * 128, num_idxs_reg=cn * 128,
                            elem_size=ROW, queue_num=0,
                        )

                    # per-window segments: [A1 | A2 | B1 | B2]; each segment's
                    # DVE chain runs as soon as its gathers land, and its
                    # matmuls overlap the next segment's chain.
                    P = pb_m.tile([128, G_MAX, 16], BF16, tag="P")
                    ev = pb_m.tile([128, G_MAX, 8], BF16, tag="ev")
                    g = pb_g.tile([128, G_MAX, HD + HEADS], BF16, tag="g")
                    pss = [pb_ps.tile([128, HD + HEADS], F32, tag=f"ps{q}",
                                      name=f"psq{q}")
                           for q in range(len(grp))]

                    def seg(lo, hi, w, ps, is_first, is_last):
                        n = hi - lo
                        if n == 0:
                            return
                        nc.vector.tensor_tensor(
                            out=P[:, lo:hi, :],
                            in0=gath[:, lo:hi, EA_OFF : EA_OFF + 16],
                            in1=aloc[:, w : w + 1, :].to_broadcast([128, n, 16]),
                            op=mybir.AluOpType.mult,
                        )
                        nc.vector.tensor_tensor(
                            out=ev[:, lo:hi, :], in0=P[:, lo:hi, 0:8],
                            in1=P[:, lo:hi, 8:16],
                            op=mybir.AluOpType.max,
                        )
                        for hh in range(HEADS):
                            nc.vector.tensor_tensor(
                                out=g[:, lo:hi, hh * OUT_DIM : (hh + 1) * OUT_DIM]
                                    .rearrange("p f (a b) -> p f a b", b=2),
                                in0=gath[:, lo:hi, hh * OUT_DIM : (hh + 1) * OUT_DIM]
                                    .rearrange("p f (a b) -> p f a b", b=2),
                                in1=ev[:, lo:hi, 2 * hh : 2 * hh + 2][:, :, None, :]
                                    .to_broadcast([128, n, OUT_DIM // 2, 2]),
                                op=mybir.AluOpType.mult,
                            )
                        nc.scalar.activation(
                            g[:, lo:hi, HD:],
                            ev[:, lo:hi, :].rearrange("p f (h b) -> p f h b",
                                                      h=HEADS)[:, :, :, 0],
                            mybir.ActivationFunctionType.Copy,
                        )
                        for f in range(lo, hi):
                            nc.tensor.matmul(
                                out=ps[:], lhsT=ident[:], rhs=g[:, f, :],
                                start=(is_first and f == lo),
                                stop=(is_last and f == hi - 1),
                            )

                    ao = 0
                    for q, (w, fa, fb) in enumerate(grp):
                        seg(ao, ao + fa, w, pss[q], True, fb == 0)
                        ao += fa
                    bo = FAg
                    for q, (w, fa, fb) in enumerate(grp):
                        seg(bo, bo + fb, w, pss[q], fa == 0, True)
                        bo += fb

                    # software pipeline: the previous group's epilogues are
                    # emitted after this group's matmuls so the DVE queue
                    # never stalls on PSUM accumulation
                    for ps_, w_ in pending:
                        emit_epilogue(ps_, w_)
                    pending = [(pss[q], grp[q][0]) for q in range(len(grp))]
                for ps_, w_ in pending:
                    emit_epilogue(ps_, w_)

                nc.vector.tensor_scalar(
                    out=y_all[:], in0=y_all[:], scalar1=bfp_rep[:, :1], scalar2=None,
                    op0=mybir.AluOpType.add,
                )
                nc.sync.dma_start(
                    out=y_d[:, :].rearrange("(k p) o -> p (k o)", p=128),
                    in_=y_all[:, :],
                )

    nc.compile()
    return nc


_CACHE = {}


def kernel(**inputs) -> np.ndarray:
    x = np.asarray(inputs["x"], dtype=np.float32)
    edge_index = np.asarray(inputs["edge_index"])
    W1 = np.asarray(inputs["W1"], dtype=np.float32)
    attn_src = np.asarray(inputs["attn_src"], dtype=np.float32)
    attn_dst = np.asarray(inputs["attn_dst"], dtype=np.float32)
    b1 = np.asarray(inputs["b1"], dtype=np.float32)
    Wf = np.asarray(inputs["Wf"], dtype=np.float32)
    bf = np.asarray(inputs["bf"], dtype=np.float32)

    # fold attention vectors through W1; duplicate-pairs layout with the 0.2x
    # scaled copies for the leaky-relu max trick
    A_src = np.zeros((HD, HEADS), dtype=np.float32)
    A_dst = np.zeros((HD, HEADS), dtype=np.float32)
    for hh in range(HEADS):
        A_src[hh * OUT_DIM : (hh + 1) * OUT_DIM, hh] = attn_src[hh]
        A_dst[hh * OUT_DIM : (hh + 1) * OUT_DIM, hh] = attn_dst[hh]
    WA_src = W1 @ A_src  # [128, 4]
    WA_dst = W1 @ A_dst

    def duppairs(WA):
        # [128,4] -> [128,16]: [a0,a0,a1,a1,a2,a2,a3,a3, .2a0,.2a0,...]
        d = np.repeat(WA, 2, axis=1)  # [128, 8]
        return np.concatenate([d, 0.2 * d], axis=1)

    w1c = np.concatenate([W1, duppairs(WA_src)], axis=1).astype(np.float32)
    w1d = duppairs(WA_dst).astype(np.float32)
    bfp = (bf.reshape(()) - Wf.sum()).reshape(1, 1).astype(np.float32)

    gspec, idx_wrapped, node_of = preprocess(edge_index)

    has_b1 = bool(np.any(b1 != 0.0))
    pkey = (gspec, has_b1)
    if pkey not in _CACHE:
        _CACHE[pkey] = build_program(gspec, HAS_B1=has_b1)
    nc = _CACHE[pkey]

    xpad = np.zeros((NODES_PAD, IN_DIM), dtype=np.float32)
    xpad[:N_NODES] = x
    xT = np.ascontiguousarray(xpad.T).astype(ml_dtypes.bfloat16)

    in_maps = []
    for c in range(N_CORES):
        nids = node_of[c].reshape(-1)  # [6272], -1 for fakes
        xmy = np.zeros((NPC_PAD, IN_DIM), dtype=np.float32)
        valid = nids >= 0
        xmy[valid] = x[nids[valid]]
        xmyT = np.ascontiguousarray(xmy.T).astype(ml_dtypes.bfloat16)
        in_maps.append(
            {
                "xT": xT,
                "xmyT": xmyT,
                "w1c": w1c,
                "w1d": w1d,
                "b1": b1.reshape(1, HD),
                "wf": Wf.reshape(HD)[None, :].astype(np.float32),
                "bfp": bfp,
                "idx": idx_wrapped[c],
            }
        )

    res = bass_utils.run_bass_kernel_spmd(nc, in_maps, core_ids=list(range(N_CORES)))
    y = np.zeros((N_NODES, 1), dtype=np.float32)
    for c in range(N_CORES):
        yc = res.results[c]["y"].reshape(-1)  # [6272]
        nids = node_of[c].reshape(-1)
        valid = nids >= 0
        y[nids[valid], 0] = yc[valid]
    return y


# revision 5
# speedup vs baseline: 1.0156x; 1.0156x over previous
"""GAT layer kernel for Trainium2, 8 NeuronCores — dst-major edge layout.

Math (exact reformulation of the reference):
  proj = x @ W1;  a = proj . attn_src (per head);  d = proj . attn_dst
  score_e = leaky_relu(a[src] + d[dst], 0.2)
  exp(leaky_relu(s)) = max(exp(s), exp(0.2 s))       [monotonicity of exp]
                     = max(ea[src]*ed[dst], ea2[src]*ed2[dst])
  with ea = exp(a), ea2 = exp(0.2 a), ed = exp(d), ed2 = exp(0.2 d).
  (The segment-max subtraction is a softmax no-op and is dropped.)
  U[n]     = sum_{dst=n} expv_e * proj[src_e];  denom[n] = sum expv_e
  y = (elu(U/denom + b1) + 1) @ Wf + (bf - sum(Wf))

Device strategy:
  Nodes are assigned to (core, window, partition) by host permutation with
  windows of 128 dst nodes of similar (degA, degB) so per-partition edge
  lists pad tightly.  Edge slot (p, f): partition p = dst, f along free dim.
  Phase A builds a DRAM node-row table [proj bf16(128) | dup-pairs of
  ea/ea2 bf16(16) | pad] (512B rows) in two tables split at node 31744 so
  gather indices fit int16; row 0 of each is an all-zero sentinel used for
  padding slots (expv = 0).
  Phase B processes windows in pairs (one big-F with one small-F rank)
  sharing a gather region [A1|A2|B1|B2] so gather calls merge across the
  pair: dma_gather rows by src, expv = max(ea*ed, ea2*ed2) with the
  per-window ed table resident in SBUF, g = proj * expv, then per-slot
  identity-weight matmuls accumulate [U | denom] into PSUM (the identity
  stays loaded in the PE array for the whole phase), then normalize, bias,
  elu, and the final 128->1 projection, software-pipelined one group back.
"""

import numpy as np
import ml_dtypes

import concourse.bacc as bacc
import concourse.tile as tile
import concourse.mybir as mybir
import concourse.bass_utils as bass_utils
from concourse.masks import make_identity

# ---------------- problem constants ----------------
N_NODES = 50000
IN_DIM = 128
HEADS = 4
OUT_DIM = 32
HD = HEADS * OUT_DIM  # 128
N_CORES = 8
N_WIN = 49                      # windows per core
WIN = 128                       # dst nodes per window (one per partition)
NPC_PAD = N_WIN * WIN           # 6272 node slots per core
N_WIN_TOT = N_CORES * N_WIN     # 392
NODES_PAD = N_WIN_TOT * WIN     # 50176
SPLIT = 31744                   # node-id split for the two tables (31 chunks)
CHUNK = 1024                    # phase-A nodes per chunk (49 chunks)
ROW = 256                       # table row, bf16 elems (512B)
EA_OFF = 128                    # row cols 128:144 hold dup-pairs ea|ea2
TBLA_ROWS = SPLIT + 2           # sentinel + nodes [0, 31744)
TBLB_ROWS = (NODES_PAD - SPLIT) + 2  # sentinel + nodes [31744, 50176)
GCHUNK = 8                      # gather tiles per call (1024-idx ucode limit)
SCRATCH = 16384                 # swdge ring bytes (default; HW-fixed)

F32 = mybir.dt.float32
BF16 = mybir.dt.bfloat16
I16 = mybir.dt.int16


def _wrap_idx(idx_flat: np.ndarray) -> np.ndarray:
    """[n] int16 -> [128, n//16] wrapped (idx i at [i%16, i//16]), replicated
    down all 128 partitions (8 gpsimd core groups x 16)."""
    n = idx_flat.shape[0]
    assert n % 16 == 0
    w = np.zeros((128, n // 16), dtype=np.int16)
    w[:16, :] = idx_flat.reshape(n // 16, 16).T
    for r in range(1, 8):
        w[16 * r : 16 * (r + 1), :] = w[:16, :]
    return w


def preprocess(edge_index: np.ndarray):
    """Structure-only preprocessing: node permutation, window profiles and
    per-core wrapped gather indices."""
    src0 = edge_index[0].astype(np.int64)
    dst0 = edge_index[1].astype(np.int64)
    loops = np.arange(N_NODES, dtype=np.int64)
    src = np.concatenate([src0, dst0, loops])
    dst = np.concatenate([dst0, src0, loops])

    isB = src >= SPLIT
    degA = np.bincount(dst[~isB], minlength=N_NODES)
    degB = np.bincount(dst[isB], minlength=N_NODES)

    # windows of 128 nodes with similar (degA, degB): tight per-partition pads
    order = np.lexsort((-degB, -degA))          # primary: degA desc
    wFA = np.zeros(N_WIN_TOT, np.int64)
    wFB = np.zeros(N_WIN_TOT, np.int64)
    dA_pad = np.concatenate([degA[order], np.zeros(NODES_PAD - N_NODES, np.int64)])
    dB_pad = np.concatenate([degB[order], np.zeros(NODES_PAD - N_NODES, np.int64)])
    wFA = dA_pad.reshape(N_WIN_TOT, WIN).max(1)
    wFB = dB_pad.reshape(N_WIN_TOT, WIN).max(1)

    # windows sorted by (FA, FB) desc; window j -> core j%8 at rank j//8.
    # Groups of 8 consecutive windows share a rank, so the per-rank common
    # profile (max over the group) should stay tight for FA and FB jointly:
    # FA-sort in blocks of 64, FB-cluster within each block, then a small
    # deterministic swap anneal.
    wo = np.lexsort((-wFB, -wFA))
    out = []
    for b0 in range(0, N_WIN_TOT, 64):
        blk = wo[b0 : b0 + 64]
        out.append(blk[np.argsort(-wFB[blk], kind="stable")])
    worder = np.concatenate(out)

    def _prof_cost(w):
        fa = wFA[w].reshape(N_WIN, 8).max(1)
        fb = wFB[w].reshape(N_WIN, 8).max(1)
        return (fa + fb).sum()

    rng = np.random.default_rng(0)
    c = _prof_cost(worder)
    for _ in range(40000):
        i, j = rng.integers(0, N_WIN_TOT, 2)
        if i // 8 == j // 8:
            continue
        worder[i], worder[j] = worder[j], worder[i]
        c2 = _prof_cost(worder)
        if c2 <= c:
            c = c2
        else:
            worder[i], worder[j] = worder[j], worder[i]
    # common profile per rank: max over the 8 cores sharing the rank
    FA_prof = np.zeros(N_WIN, np.int64)
    FB_prof = np.zeros(N_WIN, np.int64)
    for k in range(N_WIN):
        grp = worder[k * 8 : (k + 1) * 8]
        FA_prof[k] = wFA[grp].max()
        FB_prof[k] = wFB[grp].max()
    F_prof = FA_prof + FB_prof

    # node -> (core, rank k, partition p)
    # global window gw (in sorted-node order) sits at rank k=pos//8, core=pos%8
    gw_rank = np.zeros(N_WIN_TOT, np.int64)
    gw_core = np.zeros(N_WIN_TOT, np.int64)
    for j, gw in enumerate(worder):
        gw_core[gw] = j % 8
        gw_rank[gw] = j // 8

    pos = np.empty(N_NODES, np.int64)           # node -> sorted position
    pos[order] = np.arange(N_NODES)
    node_gw = pos // WIN
    node_p = pos % WIN
    node_core = gw_core[node_gw]
    node_rank = gw_rank[node_gw]

    # map (core, rank, p) -> node for aloc/xmyT and output unpermute
    node_of = np.full((N_CORES, N_WIN, WIN), -1, np.int64)
    node_of[node_core, node_rank, node_p] = np.arange(N_NODES)

    # per-edge slot assignment
    e_core = node_core[dst]
    e_rank = node_rank[dst]
    e_p = node_p[dst]
    e_isB = isB.astype(np.int64)
    # rank within (dst, class): stable sort by (dst, class)
    ek = (pos[dst] << 1) | e_isB
    es = np.argsort(ek, kind="stable")
    ek_s = ek[es]
    new_run = np.ones(len(ek_s), dtype=bool)
    new_run[1:] = ek_s[1:] != ek_s[:-1]
    run_start = np.where(new_run)[0]
    start_per = run_start[np.cumsum(new_run) - 1]
    rank_in = np.arange(len(ek_s)) - start_per
    e_f = np.empty(len(src), np.int64)
    e_f[es] = rank_in

    # ---- rank pairing: groups of 2 ranks (big F with small F) share one
    # gather region laid out [A1 | A2 | B1 | B2], merging gather calls ----
    rsort = np.argsort(-F_prof, kind="stable")
    groups = []
    for i in range(N_WIN // 2):
        groups.append((int(rsort[i]), int(rsort[N_WIN - 1 - i])))
    groups.append((int(rsort[N_WIN // 2]),))  # odd one out

    # per-rank: group base offset contributions
    abase = np.zeros(N_WIN, np.int64)   # f-offset of rank's A segment
    bbase = np.zeros(N_WIN, np.int64)   # f-offset of rank's B segment
    goff = 0
    gspec = []  # per group: (off, [(rank, FA, FB), ...])
    for grp in groups:
        FAg = sum(int(FA_prof[r]) for r in grp)
        ao = goff
        for r in grp:
            abase[r] = ao
            ao += int(FA_prof[r])
        bo = goff + FAg
        for r in grp:
            bbase[r] = bo
            bo += int(FB_prof[r])
        gspec.append((goff, tuple((int(r), int(FA_prof[r]), int(FB_prof[r]))
                                  for r in grp)))
        goff = bo
    F_tot = int(goff)

    e_f = np.where(e_isB == 1, bbase[e_rank] + e_f, abase[e_rank] + e_f)

    # idx value: A: src+1 in tblA ; B: (src-SPLIT)+1 in tblB ; sentinel 0
    e_idx = np.where(e_isB == 1, src - SPLIT + 1, src + 1).astype(np.int16)

    e_slot = e_f * WIN + e_p

    idx_blob = np.zeros((N_CORES, F_tot * WIN), dtype=np.int16)
    idx_blob[e_core, e_slot] = e_idx

    idx_wrapped = np.stack([_wrap_idx(idx_blob[c]) for c in range(N_CORES)])

    return (
        tuple(gspec),
        idx_wrapped,
        node_of,
    )


# ---------------- bass program ----------------
def build_program(gspec, HAS_B1=True):
    # gspec: per group (offset_tiles, ((rank, FA, FB), ...1-2 windows))
    G_MAX = max(sum(fa + fb for _, fa, fb in grp) for _, grp in gspec)
    last_off, last_grp = gspec[-1]
    F_tot = last_off + sum(fa + fb for _, fa, fb in last_grp)

    nc = bacc.Bacc("TRN2", target_bir_lowering=False, debug=False, num_devices=1,
                   num_swdge_queues=1, dynamic_dma_scratch_size=SCRATCH)

    xT_d = nc.dram_tensor("xT", (IN_DIM, NODES_PAD), BF16, kind="ExternalInput").ap()
    xmyT_d = nc.dram_tensor("xmyT", (IN_DIM, NPC_PAD), BF16, kind="ExternalInput").ap()
    w1c_d = nc.dram_tensor("w1c", (IN_DIM, HD + 16), F32, kind="ExternalInput").ap()
    w1d_d = nc.dram_tensor("w1d", (IN_DIM, 16), F32, kind="ExternalInput").ap()
    b1_d = nc.dram_tensor("b1", (1, HD), F32, kind="ExternalInput").ap()
    wf_d = nc.dram_tensor("wf", (1, HD), F32, kind="ExternalInput").ap()
    bfp_d = nc.dram_tensor("bfp", (1, 1), F32, kind="ExternalInput").ap()
    idx_d = nc.dram_tensor("idx", (128, F_tot * 8), I16, kind="ExternalInput").ap()
    y_d = nc.dram_tensor("y", (NPC_PAD, 1), F32, kind="ExternalOutput").ap()
    tblA_d = nc.dram_tensor("tblA", (TBLA_ROWS, ROW), BF16, kind="Internal").ap()
    tblB_d = nc.dram_tensor("tblB", (TBLB_ROWS, ROW), BF16, kind="Internal").ap()

    with tile.TileContext(nc) as tc:
        with tc.tile_pool(name="const", bufs=1) as consts:
            ident = consts.tile([128, 128], BF16)
            make_identity(nc, ident[:])
            w1cf = consts.tile([128, HD + 16], F32)
            nc.sync.dma_start(out=w1cf[:], in_=w1c_d[:, :])
            w1c = consts.tile([128, HD + 16], BF16)
            nc.vector.tensor_copy(out=w1c[:], in_=w1cf[:])
            w1df = consts.tile([128, 16], F32)
            nc.sync.dma_start(out=w1df[:], in_=w1d_d[:, :])
            w1d = consts.tile([128, 16], BF16)
            nc.vector.tensor_copy(out=w1d[:], in_=w1df[:])

            # zero sentinels (row 0 of each table)
            zrow = consts.tile([1, ROW], BF16)
            nc.vector.memset(zrow[:], 0.0)
            nc.sync.dma_start(out=tblA_d[0:1, :], in_=zrow[:])
            nc.sync.dma_start(out=tblB_d[0:1, :], in_=zrow[:])

            # replicated epilogue params via ones-matmul
            ones_row = consts.tile([1, 128], F32)
            nc.vector.memset(ones_row[:], 1.0)
            b1_t = consts.tile([1, HD], F32)
            nc.sync.dma_start(out=b1_t[:], in_=b1_d[:, :])
            wf_t = consts.tile([1, HD], F32)
            nc.sync.dma_start(out=wf_t[:], in_=wf_d[:, :])
            bfp_t = consts.tile([1, 1], F32)
            nc.sync.dma_start(out=bfp_t[:], in_=bfp_d[:, :])
            with tc.tile_pool(name="bc_ps", bufs=1, space="PSUM") as bc_ps:
                bput = bc_ps.tile([128, 2 * HD + 1], F32)
                nc.tensor.matmul(out=bput[:, :HD], lhsT=ones_row[:, :], rhs=b1_t[:, :],
                                 start=True, stop=False)
                nc.tensor.matmul(out=bput[:, HD : 2 * HD], lhsT=ones_row[:, :],
                                 rhs=wf_t[:, :], start=False, stop=False)
                nc.tensor.matmul(out=bput[:, 2 * HD :], lhsT=ones_row[:, :],
                                 rhs=bfp_t[:, :], start=False, stop=True)
                b1_rep = consts.tile([128, HD], F32)
                nc.vector.tensor_copy(out=b1_rep[:], in_=bput[:, :HD])
                wf_rep = consts.tile([128, HD], F32)
                nc.vector.tensor_copy(out=wf_rep[:], in_=bput[:, HD : 2 * HD])
                bfp_rep = consts.tile([128, 1], F32)
                nc.vector.tensor_copy(out=bfp_rep[:], in_=bput[:, 2 * HD :])

            aloc = consts.tile([128, N_WIN, 16], BF16)
            y_all = consts.tile([128, N_WIN], F32)

            # idx blob: prefetch the whole thing into SBUF once
            idx_sb = consts.tile([128, F_tot * 8], I16)
            nc.sync.dma_start(out=idx_sb[:], in_=idx_d[:, :])

            # ---- phase A': per-core dst exp table (SBUF-resident) ----
            with (
                tc.tile_pool(name="pd_x", bufs=2) as pd_x,
                tc.tile_pool(name="pd_ps", bufs=2, space="PSUM") as pd_ps,
            ):
                xmt = pd_x.tile([128, NPC_PAD], BF16, tag="xmt")
                nc.sync.dma_start(out=xmt[:], in_=xmyT_d[:, :])
                for k in range(N_WIN):
                    ps = pd_ps.tile([128, 16], F32, tag="psd")
                    nc.tensor.matmul(
                        out=ps[:], lhsT=xmt[:, k * 128 : (k + 1) * 128],
                        rhs=w1d[:, :], start=True, stop=True,
                    )
                    nc.scalar.activation(aloc[:, k, :], ps[:],
                                         mybir.ActivationFunctionType.Exp)

            # ---- phase A: node tables (chunks of 1024 nodes) ----
            # manual row buffers: pad cols initialized once, full-row stores
            NROW = 5
            rows = [consts.tile([128, 8, ROW], BF16, name=f"rowbuf{i}")
                    for i in range(NROW)]
            for r in rows:
                nc.vector.memset(r[:, :, EA_OFF + 16 :], 0.0)
            with (
                tc.tile_pool(name="pa_x", bufs=6) as pa_x,
                tc.tile_pool(name="pa_ps", bufs=8, space="PSUM") as pa_ps,
            ):
                for c in range(49):
                    xt = pa_x.tile([128, CHUNK], BF16, tag="xt")
                    nc.sync.dma_start(out=xt[:], in_=xT_d[:, c * CHUNK : (c + 1) * CHUNK])
                    row = rows[c % NROW]
                    for i in range(4):  # pairs of 128-node tiles
                        ps = pa_ps.tile([128, 2, HD + 16], F32, tag="ps")
                        for j in range(2):
                            t = 2 * i + j
                            nc.tensor.matmul(
                                out=ps[:, j, :], lhsT=xt[:, t * 128 : (t + 1) * 128],
                                rhs=w1c[:, :], start=True, stop=True,
                            )
                        if i < 3:
                            nc.vector.tensor_copy(
                                out=row[:, 2 * i : 2 * i + 2, :HD], in_=ps[:, :, :HD])
                        else:
                            nc.scalar.activation(
                                row[:, 2 * i : 2 * i + 2, :HD], ps[:, :, :HD],
                                mybir.ActivationFunctionType.Copy)
                        nc.scalar.activation(row[:, 2 * i : 2 * i + 2,
                                                 EA_OFF : EA_OFF + 16],
                                             ps[:, :, HD : HD + 16],
                                             mybir.ActivationFunctionType.Exp)
                    if c < 31:
                        dst_ap = tblA_d[1 + c * CHUNK : 1 + (c + 1) * CHUNK, :]
                    else:
                        c2 = c - 31
                        dst_ap = tblB_d[1 + c2 * CHUNK : 1 + (c2 + 1) * CHUNK, :]
                    nc.sync.dma_start(
                        out=dst_ap.rearrange("(t p) e -> p t e", p=128),
                        in_=row[:],
                    )

            # ---- phase B ----
            with (
                tc.tile_pool(name="pb_gath", bufs=2) as pb_gath,
                tc.tile_pool(name="pb_g", bufs=2) as pb_g,
                tc.tile_pool(name="pb_m", bufs=3) as pb_m,
                tc.tile_pool(name="pb_s", bufs=3) as pb_s,
                tc.tile_pool(name="pb_ps", bufs=4, space="PSUM") as pb_ps,
            ):
                def emit_epilogue(ps, w):
                    dn = pb_s.tile([128, HEADS], F32, tag="dn")
                    nc.vector.tensor_scalar_max(dn[:], ps[:, HD:], 1e-12)
                    rec = pb_s.tile([128, HEADS, 1], F32, tag="rec")
                    nc.vector.reciprocal(rec[:, :, 0], dn[:])
                    h = pb_s.tile([128, HD], F32, tag="h")
                    nc.vector.tensor_tensor(
                        out=h[:],
                        in0=ps[:, :HD],
                        in1=rec[:, :, :1].to_broadcast([128, HEADS, OUT_DIM]),
                        op=mybir.AluOpType.mult,
                    )
                    if HAS_B1:
                        nc.vector.tensor_tensor(out=h[:], in0=h[:], in1=b1_rep[:],
                                                op=mybir.AluOpType.add)
                    # elu(h)+1 = exp(min(h,0)) + relu(h)  (-1 folded into bfp)
                    hm = pb_s.tile([128, HD], F32, tag="hm")
                    nc.vector.tensor_scalar_min(hm[:], h[:], 0.0)
                    em = pb_s.tile([128, HD], F32, tag="em")
                    nc.scalar.activation(em[:], hm[:],
                                         mybir.ActivationFunctionType.Exp)
                    rl = pb_s.tile([128, HD], F32, tag="rl")
                    nc.scalar.activation(rl[:], h[:],
                                         mybir.ActivationFunctionType.Relu)
                    e = pb_s.tile([128, HD], F32, tag="e")
                    nc.vector.tensor_tensor(out=e[:], in0=em[:], in1=rl[:],
                                            op=mybir.AluOpType.add)
                    scr = pb_s.tile([128, HD], F32, tag="scr")
                    nc.vector.tensor_tensor(out=scr[:], in0=e[:], in1=wf_rep[:],
                                            op=mybir.AluOpType.mult)
                    nc.vector.tensor_reduce(
                        out=y_all[:, w : w + 1], in_=scr[:],
                        axis=mybir.AxisListType.X, op=mybir.AluOpType.add,
                    )

                pending = []  # [(ps, rank), ...] epilogues of the previous group
                for goff, grp in gspec:
                    FAg = sum(fa for _, fa, _ in grp)
                    Fg = FAg + sum(fb for _, _, fb in grp)
                    o8 = goff * 8

                    gath = pb_gath.tile([128, G_MAX, ROW], BF16, tag="gath")
                    for c0 in range(0, FAg, GCHUNK):
                        cn = min(GCHUNK, FAg - c0)
                        nc.gpsimd.dma_gather(
                            out_ap=gath[:, c0 : c0 + cn, :], in_ap=tblA_d[:, :],
                            idxs_ap=idx_sb[:, o8 + c0 * 8 : o8 + (c0 + cn) * 8],
                            num_idxs=cn * 128, num_idxs_reg=cn * 128,
                            elem_size=ROW, queue_num=0,
                        )
                    for c0 in range(FAg, Fg, GCHUNK):
                        cn = min(GCHUNK, Fg - c0)
                        nc.gpsimd.dma_gather(
                            out_ap=gath[:, c0 : c0 + cn, :],
                            in_ap=tblB_d[:, :],
                            idxs_ap=idx_sb[:, o8 + c0 * 8 : o8 + (c0 + cn) * 8],
                            num_idxs=cn * 128, num_idxs_reg=cn * 128,
                            elem_size=ROW, queue_num=0,
                        )

                    # per-window segments: [A1 | A2 | B1 | B2]; each segment's
                    # DVE chain runs as soon as its gathers land, and its
                    # matmuls overlap the next segment's chain.
                    P = pb_m.tile([128, G_MAX, 16], BF16, tag="P")
                    ev = pb_m.tile([128, G_MAX, 8], BF16, tag="ev")
                    g = pb_g.tile([128, G_MAX, HD + HEADS], BF16, tag="g")
                    pss = [pb_ps.tile([128, HD + HEADS], F32, tag=f"ps{q}",
                                      name=f"psq{q}")
                           for q in range(len(grp))]

                    def seg(lo, hi, w, ps, is_first, is_last):
                        n = hi - lo
                        if n == 0:
                            return
                        nc.vector.tensor_tensor(
                            out=P[:, lo:hi, :],
                            in0=gath[:, lo:hi, EA_OFF : EA_OFF + 16],
                            in1=aloc[:, w : w + 1, :].to_broadcast([128, n, 16]),
                            op=mybir.AluOpType.mult,
                        )
                        nc.vector.tensor_tensor(
                            out=ev[:, lo:hi, :], in0=P[:, lo:hi, 0:8],
                            in1=P[:, lo:hi, 8:16],
                            op=mybir.AluOpType.max,
                        )
                        for hh in range(HEADS):
                            nc.vector.tensor_tensor(
                                out=g[:, lo:hi, hh * OUT_DIM : (hh + 1) * OUT_DIM]
                                    .rearrange("p f (a b) -> p f a b", b=2),
                                in0=gath[:, lo:hi, hh * OUT_DIM : (hh + 1) * OUT_DIM]
                                    .rearrange("p f (a b) -> p f a b", b=2),
                                in1=ev[:, lo:hi, 2 * hh : 2 * hh + 2][:, :, None, :]
                                    .to_broadcast([128, n, OUT_DIM // 2, 2]),
                                op=mybir.AluOpType.mult,
                            )
                        nc.scalar.activation(
                            g[:, lo:hi, HD:],
                            ev[:, lo:hi, :].rearrange("p f (h b) -> p f h b",
                                                      h=HEADS)[:, :, :, 0],
                            mybir.ActivationFunctionType.Copy,
                        )
                        for f in range(lo, hi):
                            nc.tensor.matmul(
                                out=ps[:], lhsT=ident[:], rhs=g[:, f, :],
                                start=(is_first and f == lo),
                                stop=(is_last and f == hi - 1),
                            )

                    ao = 0
                    for q, (w, fa, fb) in enumerate(grp):
                        seg(ao, ao + fa, w, pss[q], True, fb == 0)
                        ao += fa
                    bo = FAg
                    for q, (w, fa, fb) in enumerate(grp):
                        seg(bo, bo + fb, w, pss[q], fa == 0, True)
                        bo += fb

                    # software pipeline: the previous group's epilogues are
                    # emitted after this group's matmuls so the DVE queue
                    # never stalls on PSUM accumulation
                    for ps_, w_ in pending:
                        emit_epilogue(ps_, w_)
                    pending = [(pss[q], grp[q][0]) for q in range(len(grp))]
                for ps_, w_ in pending:
                    emit_epilogue(ps_, w_)

                nc.vector.tensor_scalar(
                    out=y_all[:], in0=y_all[:], scalar1=bfp_rep[:, :1], scalar2=None,
                    op0=mybir.AluOpType.add,
                )
                nc.sync.dma_start(
                    out=y_d[:, :].rearrange("(k p) o -> p (k o)", p=128),
                    in_=y_all[:, :],
                )

    nc.compile()
    return nc


_CACHE = {}


def kernel(**inputs) -> np.ndarray:
    x = np.asarray(inputs["x"], dtype=np.float32)
    edge_index = np.asarray(inputs["edge_index"])
    W1 = np.asarray(inputs["W1"], dtype=np.float32)
    attn_src = np.asarray(inputs["attn_src"], dtype=np.float32)
    attn_dst = np.asarray(inputs["attn_dst"], dtype=np.float32)
    b1 = np.asarray(inputs["b1"], dtype=np.float32)
    Wf = np.asarray(inputs["Wf"], dtype=np.float32)
    bf = np.asarray(inputs["bf"], dtype=np.float32)

    # fold attention vectors through W1; duplicate-pairs layout with the 0.2x
    # scaled copies for the leaky-relu max trick
    A_src = np.zeros((HD, HEADS), dtype=np.float32)
    A_dst = np.zeros((HD, HEADS), dtype=np.float32)
    for hh in range(HEADS):
        A_src[hh * OUT_DIM : (hh + 1) * OUT_DIM, hh] = attn_src[hh]
        A_dst[hh * OUT_DIM : (hh + 1) * OUT_DIM, hh] = attn_dst[hh]
    WA_src = W1 @ A_src  # [128, 4]
    WA_dst = W1 @ A_dst

    def duppairs(WA):
        # [128,4] -> [128,16]: [a0,a0,a1,a1,a2,a2,a3,a3, .2a0,.2a0,...]
        d = np.repeat(WA, 2, axis=1)  # [128, 8]
        return np.concatenate([d, 0.2 * d], axis=1)

    w1c = np.concatenate([W1, duppairs(WA_src)], axis=1).astype(np.float32)
    w1d = duppairs(WA_dst).astype(np.float32)
    bfp = (bf.reshape(()) - Wf.sum()).reshape(1, 1).astype(np.float32)

    gspec, idx_wrapped, node_of = preprocess(edge_index)

    has_b1 = bool(np.any(b1 != 0.0))
    pkey = (gspec, has_b1)
    if pkey not in _CACHE:
        _CACHE[pkey] = build_program(gspec, HAS_B1=has_b1)
    nc = _CACHE[pkey]

    xpad = np.zeros((NODES_PAD, IN_DIM), dtype=np.float32)
    xpad[:N_NODES] = x
    xT = np.ascontiguousarray(xpad.T).astype(ml_dtypes.bfloat16)

    in_maps = []
    for c in range(N_CORES):
        nids = node_of[c].reshape(-1)  # [6272], -1 for fakes
        xmy = np.zeros((NPC_PAD, IN_DIM), dtype=np.float32)
        valid = nids >= 0
        xmy[valid] = x[nids[valid]]
        xmyT = np.ascontiguousarray(xmy.T).astype(ml_dtypes.bfloat16)
        in_maps.append(
            {
                "xT": xT,
                "xmyT": xmyT,
                "w1c": w1c,
                "w1d": w1d,
                "b1": b1.reshape(1, HD),
                "wf": Wf.reshape(HD)[None, :].astype(np.float32),
                "bfp": bfp,
                "idx": idx_wrapped[c],
            }
        )

    res = bass_utils.run_bass_kernel_spmd(nc, in_maps, core_ids=list(range(N_CORES)))
    y = np.zeros((N_NODES, 1), dtype=np.float32)
    for c in range(N_CORES):
        yc = res.results[c]["y"].reshape(-1)  # [6272]
        nids = node_of[c].reshape(-1)
        valid = nids >= 0
        y[nids[valid], 0] = yc[valid]
    return y


# revision 6
# speedup vs baseline: 1.0176x; 1.0020x over previous
"""GAT layer kernel for Trainium2, 8 NeuronCores — dst-major edge layout.

Math (exact reformulation of the reference):
  proj = x @ W1;  a = proj . attn_src (per head);  d = proj . attn_dst
  score_e = leaky_relu(a[src] + d[dst], 0.2)
  exp(leaky_relu(s)) = max(exp(s), exp(0.2 s))       [monotonicity of exp]
                     = max(ea[src]*ed[dst], ea2[src]*ed2[dst])
  with ea = exp(a), ea2 = exp(0.2 a), ed = exp(d), ed2 = exp(0.2 d).
  (The segment-max subtraction is a softmax no-op and is dropped.)
  U[n]     = sum_{dst=n} expv_e * proj[src_e];  denom[n] = sum expv_e
  y = (elu(U/denom + b1) + 1) @ Wf + (bf - sum(Wf))

Device strategy:
  Nodes are assigned to (core, window, partition) by host permutation with
  windows of 128 dst nodes of similar (degA, degB) so per-partition edge
  lists pad tightly.  Edge slot (p, f): partition p = dst, f along free dim.
  Phase A builds a DRAM node-row table [proj bf16(128) | dup-pairs of
  ea/ea2 bf16(16) | pad] (512B rows) in two tables split at node 31744 so
  gather indices fit int16; row 0 of each is an all-zero sentinel used for
  padding slots (expv = 0).
  Phase B processes windows in pairs (one big-F with one small-F rank)
  sharing a gather region [A1|A2|B1|B2] so gather calls merge across the
  pair: dma_gather rows by src, expv = max(ea*ed, ea2*ed2) with the
  per-window ed table resident in SBUF, g = proj * expv, then per-slot
  identity-weight matmuls accumulate [U | denom] into PSUM (the identity
  stays loaded in the PE array for the whole phase), then normalize, bias,
  elu, and the final 128->1 projection, software-pipelined one group back.
"""

import numpy as np
import ml_dtypes

import concourse.bacc as bacc
import concourse.tile as tile
import concourse.mybir as mybir
import concourse.bass_utils as bass_utils
from concourse.masks import make_identity

# ---------------- problem constants ----------------
N_NODES = 50000
IN_DIM = 128
HEADS = 4
OUT_DIM = 32
HD = HEADS * OUT_DIM  # 128
N_CORES = 8
N_WIN = 49                      # windows per core
WIN = 128                       # dst nodes per window (one per partition)
NPC_PAD = N_WIN * WIN           # 6272 node slots per core
N_WIN_TOT = N_CORES * N_WIN     # 392
NODES_PAD = N_WIN_TOT * WIN     # 50176
SPLIT = 31744                   # node-id split for the two tables (31 chunks)
CHUNK = 1024                    # phase-A nodes per chunk (49 chunks)
ROW = 256                       # table row, bf16 elems (512B)
EA_OFF = 128                    # row cols 128:144 hold dup-pairs ea|ea2
TBLA_ROWS = SPLIT + 2           # sentinel + nodes [0, 31744)
TBLB_ROWS = (NODES_PAD - SPLIT) + 2  # sentinel + nodes [31744, 50176)
GCHUNK = 8                      # gather tiles per call (1024-idx ucode limit)
SCRATCH = 16384                 # swdge ring bytes (default; HW-fixed)

F32 = mybir.dt.float32
BF16 = mybir.dt.bfloat16
I16 = mybir.dt.int16


def _wrap_idx(idx_flat: np.ndarray) -> np.ndarray:
    """[n] int16 -> [128, n//16] wrapped (idx i at [i%16, i//16]), replicated
    down all 128 partitions (8 gpsimd core groups x 16)."""
    n = idx_flat.shape[0]
    assert n % 16 == 0
    w = np.zeros((128, n // 16), dtype=np.int16)
    w[:16, :] = idx_flat.reshape(n // 16, 16).T
    for r in range(1, 8):
        w[16 * r : 16 * (r + 1), :] = w[:16, :]
    return w


def preprocess(edge_index: np.ndarray):
    """Structure-only preprocessing: node permutation, window profiles and
    per-core wrapped gather indices."""
    src0 = edge_index[0].astype(np.int64)
    dst0 = edge_index[1].astype(np.int64)
    loops = np.arange(N_NODES, dtype=np.int64)
    src = np.concatenate([src0, dst0, loops])
    dst = np.concatenate([dst0, src0, loops])

    isB = src >= SPLIT
    degA = np.bincount(dst[~isB], minlength=N_NODES)
    degB = np.bincount(dst[isB], minlength=N_NODES)

    # windows of 128 nodes with similar (degA, degB): tight per-partition pads
    order = np.lexsort((-degB, -degA))          # primary: degA desc
    wFA = np.zeros(N_WIN_TOT, np.int64)
    wFB = np.zeros(N_WIN_TOT, np.int64)
    dA_pad = np.concatenate([degA[order], np.zeros(NODES_PAD - N_NODES, np.int64)])
    dB_pad = np.concatenate([degB[order], np.zeros(NODES_PAD - N_NODES, np.int64)])
    wFA = dA_pad.reshape(N_WIN_TOT, WIN).max(1)
    wFB = dB_pad.reshape(N_WIN_TOT, WIN).max(1)

    # windows sorted by (FA, FB) desc; window j -> core j%8 at rank j//8.
    # Groups of 8 consecutive windows share a rank, so the per-rank common
    # profile (max over the group) should stay tight for FA and FB jointly:
    # FA-sort in blocks of 64, FB-cluster within each block, then a small
    # deterministic swap anneal.
    wo = np.lexsort((-wFB, -wFA))
    out = []
    for b0 in range(0, N_WIN_TOT, 64):
        blk = wo[b0 : b0 + 64]
        out.append(blk[np.argsort(-wFB[blk], kind="stable")])
    worder = np.concatenate(out)

    def _prof_cost(w):
        fa = wFA[w].reshape(N_WIN, 8).max(1)
        fb = wFB[w].reshape(N_WIN, 8).max(1)
        return (fa + fb).sum()

    rng = np.random.default_rng(0)
    c = _prof_cost(worder)
    for _ in range(40000):
        i, j = rng.integers(0, N_WIN_TOT, 2)
        if i // 8 == j // 8:
            continue
        worder[i], worder[j] = worder[j], worder[i]
        c2 = _prof_cost(worder)
        if c2 <= c:
            c = c2
        else:
            worder[i], worder[j] = worder[j], worder[i]
    # common profile per rank: max over the 8 cores sharing the rank
    FA_prof = np.zeros(N_WIN, np.int64)
    FB_prof = np.zeros(N_WIN, np.int64)
    for k in range(N_WIN):
        grp = worder[k * 8 : (k + 1) * 8]
        FA_prof[k] = wFA[grp].max()
        FB_prof[k] = wFB[grp].max()
    F_prof = FA_prof + FB_prof

    # node -> (core, rank k, partition p)
    # global window gw (in sorted-node order) sits at rank k=pos//8, core=pos%8
    gw_rank = np.zeros(N_WIN_TOT, np.int64)
    gw_core = np.zeros(N_WIN_TOT, np.int64)
    for j, gw in enumerate(worder):
        gw_core[gw] = j % 8
        gw_rank[gw] = j // 8

    pos = np.empty(N_NODES, np.int64)           # node -> sorted position
    pos[order] = np.arange(N_NODES)
    node_gw = pos // WIN
    node_p = pos % WIN
    node_core = gw_core[node_gw]
    node_rank = gw_rank[node_gw]

    # map (core, rank, p) -> node for aloc/xmyT and output unpermute
    node_of = np.full((N_CORES, N_WIN, WIN), -1, np.int64)
    node_of[node_core, node_rank, node_p] = np.arange(N_NODES)

    # per-edge slot assignment
    e_core = node_core[dst]
    e_rank = node_rank[dst]
    e_p = node_p[dst]
    e_isB = isB.astype(np.int64)
    # rank within (dst, class): stable sort by (dst, class)
    ek = (pos[dst] << 1) | e_isB
    es = np.argsort(ek, kind="stable")
    ek_s = ek[es]
    new_run = np.ones(len(ek_s), dtype=bool)
    new_run[1:] = ek_s[1:] != ek_s[:-1]
    run_start = np.where(new_run)[0]
    start_per = run_start[np.cumsum(new_run) - 1]
    rank_in = np.arange(len(ek_s)) - start_per
    e_f = np.empty(len(src), np.int64)
    e_f[es] = rank_in

    # ---- rank pairing: groups of 2 ranks (big F with small F) share one
    # gather region laid out [A1 | A2 | B1 | B2], merging gather calls ----
    rsort = np.argsort(-F_prof, kind="stable")
    groups = []
    for i in range(N_WIN // 2):
        groups.append((int(rsort[i]), int(rsort[N_WIN - 1 - i])))
    # smallest (solo) group first: fills the pipeline quickly after the
    # phase-A barrier; the remaining pairs big-to-small so the drain is short
    groups = [(int(rsort[N_WIN // 2]),)] + groups

    # per-rank: group base offset contributions
    abase = np.zeros(N_WIN, np.int64)   # f-offset of rank's A segment
    bbase = np.zeros(N_WIN, np.int64)   # f-offset of rank's B segment
    goff = 0
    gspec = []  # per group: (off, [(rank, FA, FB), ...])
    for grp in groups:
        FAg = sum(int(FA_prof[r]) for r in grp)
        ao = goff
        for r in grp:
            abase[r] = ao
            ao += int(FA_prof[r])
        bo = goff + FAg
        for r in grp:
            bbase[r] = bo
            bo += int(FB_prof[r])
        gspec.append((goff, tuple((int(r), int(FA_prof[r]), int(FB_prof[r]))
                                  for r in grp)))
        goff = bo
    F_tot = int(goff)

    e_f = np.where(e_isB == 1, bbase[e_rank] + e_f, abase[e_rank] + e_f)

    # idx value: A: src+1 in tblA ; B: (src-SPLIT)+1 in tblB ; sentinel 0
    e_idx = np.where(e_isB == 1, src - SPLIT + 1, src + 1).astype(np.int16)

    e_slot = e_f * WIN + e_p

    idx_blob = np.zeros((N_CORES, F_tot * WIN), dtype=np.int16)
    idx_blob[e_core, e_slot] = e_idx

    idx_wrapped = np.stack([_wrap_idx(idx_blob[c]) for c in range(N_CORES)])

    return (
        tuple(gspec),
        idx_wrapped,
        node_of,
    )


# ---------------- bass program ----------------
def build_program(gspec, HAS_B1=True):
    # gspec: per group (offset_tiles, ((rank, FA, FB), ...1-2 windows))
    G_MAX = max(sum(fa + fb for _, fa, fb in grp) for _, grp in gspec)
    last_off, last_grp = gspec[-1]
    F_tot = last_off + sum(fa + fb for _, fa, fb in last_grp)

    nc = bacc.Bacc("TRN2", target_bir_lowering=False, debug=False, num_devices=1,
                   num_swdge_queues=1, dynamic_dma_scratch_size=SCRATCH)

    xT_d = nc.dram_tensor("xT", (IN_DIM, NODES_PAD), BF16, kind="ExternalInput").ap()
    xmyT_d = nc.dram_tensor("xmyT", (IN_DIM, NPC_PAD), BF16, kind="ExternalInput").ap()
    w1c_d = nc.dram_tensor("w1c", (IN_DIM, HD + 16), F32, kind="ExternalInput").ap()
    w1d_d = nc.dram_tensor("w1d", (IN_DIM, 16), F32, kind="ExternalInput").ap()
    b1_d = nc.dram_tensor("b1", (1, HD), F32, kind="ExternalInput").ap()
    wf_d = nc.dram_tensor("wf", (1, HD), F32, kind="ExternalInput").ap()
    bfp_d = nc.dram_tensor("bfp", (1, 1), F32, kind="ExternalInput").ap()
    idx_d = nc.dram_tensor("idx", (128, F_tot * 8), I16, kind="ExternalInput").ap()
    y_d = nc.dram_tensor("y", (NPC_PAD, 1), F32, kind="ExternalOutput").ap()
    tblA_d = nc.dram_tensor("tblA", (TBLA_ROWS, ROW), BF16, kind="Internal").ap()
    tblB_d = nc.dram_tensor("tblB", (TBLB_ROWS, ROW), BF16, kind="Internal").ap()

    with tile.TileContext(nc) as tc:
        with tc.tile_pool(name="const", bufs=1) as consts:
            ident = consts.tile([128, 128], BF16)
            make_identity(nc, ident[:])
            w1cf = consts.tile([128, HD + 16], F32)
            nc.sync.dma_start(out=w1cf[:], in_=w1c_d[:, :])
            w1c = consts.tile([128, HD + 16], BF16)
            nc.vector.tensor_copy(out=w1c[:], in_=w1cf[:])
            w1df = consts.tile([128, 16], F32)
            nc.sync.dma_start(out=w1df[:], in_=w1d_d[:, :])
            w1d = consts.tile([128, 16], BF16)
            nc.vector.tensor_copy(out=w1d[:], in_=w1df[:])

            # zero sentinels (row 0 of each table)
            zrow = consts.tile([1, ROW], BF16)
            nc.vector.memset(zrow[:], 0.0)
            nc.sync.dma_start(out=tblA_d[0:1, :], in_=zrow[:])
            nc.sync.dma_start(out=tblB_d[0:1, :], in_=zrow[:])

            # replicated epilogue params via ones-matmul
            ones_row = consts.tile([1, 128], F32)
            nc.vector.memset(ones_row[:], 1.0)
            b1_t = consts.tile([1, HD], F32)
            nc.sync.dma_start(out=b1_t[:], in_=b1_d[:, :])
            wf_t = consts.tile([1, HD], F32)
            nc.sync.dma_start(out=wf_t[:], in_=wf_d[:, :])
            bfp_t = consts.tile([1, 1], F32)
            nc.sync.dma_start(out=bfp_t[:], in_=bfp_d[:, :])
            with tc.tile_pool(name="bc_ps", bufs=1, space="PSUM") as bc_ps:
                bput = bc_ps.tile([128, 2 * HD + 1], F32)
                nc.tensor.matmul(out=bput[:, :HD], lhsT=ones_row[:, :], rhs=b1_t[:, :],
                                 start=True, stop=False)
                nc.tensor.matmul(out=bput[:, HD : 2 * HD], lhsT=ones_row[:, :],
                                 rhs=wf_t[:, :], start=False, stop=False)
                nc.tensor.matmul(out=bput[:, 2 * HD :], lhsT=ones_row[:, :],
                                 rhs=bfp_t[:, :], start=False, stop=True)
                b1_rep = consts.tile([128, HD], F32)
                nc.vector.tensor_copy(out=b1_rep[:], in_=bput[:, :HD])
                wf_rep = consts.tile([128, HD], F32)
                nc.vector.tensor_copy(out=wf_rep[:], in_=bput[:, HD : 2 * HD])
                bfp_rep = consts.tile([128, 1], F32)
                nc.vector.tensor_copy(out=bfp_rep[:], in_=bput[:, 2 * HD :])

            aloc = consts.tile([128, N_WIN, 16], BF16)
            y_all = consts.tile([128, N_WIN], F32)

            # idx blob: prefetch the whole thing into SBUF once
            idx_sb = consts.tile([128, F_tot * 8], I16)
            nc.sync.dma_start(out=idx_sb[:], in_=idx_d[:, :])

            # ---- phase A': per-core dst exp table (SBUF-resident) ----
            with (
                tc.tile_pool(name="pd_x", bufs=2) as pd_x,
                tc.tile_pool(name="pd_ps", bufs=2, space="PSUM") as pd_ps,
            ):
                xmt = pd_x.tile([128, NPC_PAD], BF16, tag="xmt")
                nc.sync.dma_start(out=xmt[:], in_=xmyT_d[:, :])
                for k in range(N_WIN):
                    ps = pd_ps.tile([128, 16], F32, tag="psd")
                    nc.tensor.matmul(
                        out=ps[:], lhsT=xmt[:, k * 128 : (k + 1) * 128],
                        rhs=w1d[:, :], start=True, stop=True,
                    )
                    nc.scalar.activation(aloc[:, k, :], ps[:],
                                         mybir.ActivationFunctionType.Exp)

            # ---- phase A: node tables (chunks of 1024 nodes) ----
            # manual row buffers: pad cols initialized once, full-row stores
            NROW = 5
            rows = [consts.tile([128, 8, ROW], BF16, name=f"rowbuf{i}")
                    for i in range(NROW)]
            for r in rows:
                nc.vector.memset(r[:, :, EA_OFF + 16 :], 0.0)
            with (
                tc.tile_pool(name="pa_x", bufs=8) as pa_x,
                tc.tile_pool(name="pa_ps", bufs=8, space="PSUM") as pa_ps,
            ):
                for c in range(49):
                    xt = pa_x.tile([128, CHUNK], BF16, tag="xt")
                    nc.sync.dma_start(out=xt[:], in_=xT_d[:, c * CHUNK : (c + 1) * CHUNK])
                    row = rows[c % NROW]
                    for i in range(4):  # pairs of 128-node tiles
                        ps = pa_ps.tile([128, 2, HD + 16], F32, tag="ps")
                        for j in range(2):
                            t = 2 * i + j
                            nc.tensor.matmul(
                                out=ps[:, j, :], lhsT=xt[:, t * 128 : (t + 1) * 128],
                                rhs=w1c[:, :], start=True, stop=True,
                            )
                        if i < 3:
                            nc.vector.tensor_copy(
                                out=row[:, 2 * i : 2 * i + 2, :HD], in_=ps[:, :, :HD])
                        else:
                            nc.scalar.activation(
                                row[:, 2 * i : 2 * i + 2, :HD], ps[:, :, :HD],
                                mybir.ActivationFunctionType.Copy)
                        nc.scalar.activation(row[:, 2 * i : 2 * i + 2,
                                                 EA_OFF : EA_OFF + 16],
                                             ps[:, :, HD : HD + 16],
                                             mybir.ActivationFunctionType.Exp)
                    if c < 31:
                        dst_ap = tblA_d[1 + c * CHUNK : 1 + (c + 1) * CHUNK, :]
                    else:
                        c2 = c - 31
                        dst_ap = tblB_d[1 + c2 * CHUNK : 1 + (c2 + 1) * CHUNK, :]
                    nc.sync.dma_start(
                        out=dst_ap.rearrange("(t p) e -> p t e", p=128),
                        in_=row[:],
                    )

            # ---- phase B ----
            with (
                tc.tile_pool(name="pb_gath", bufs=2) as pb_gath,
                tc.tile_pool(name="pb_g", bufs=2) as pb_g,
                tc.tile_pool(name="pb_m", bufs=3) as pb_m,
                tc.tile_pool(name="pb_s", bufs=3) as pb_s,
                tc.tile_pool(name="pb_ps", bufs=4, space="PSUM") as pb_ps,
            ):
                def emit_epilogue(ps, w):
                    dn = pb_s.tile([128, HEADS], F32, tag="dn")
                    nc.vector.tensor_scalar_max(dn[:], ps[:, HD:], 1e-12)
                    rec = pb_s.tile([128, HEADS, 1], F32, tag="rec")
                    nc.vector.reciprocal(rec[:, :, 0], dn[:])
                    h = pb_s.tile([128, HD], F32, tag="h")
                    nc.vector.tensor_tensor(
                        out=h[:],
                        in0=ps[:, :HD],
                        in1=rec[:, :, :1].to_broadcast([128, HEADS, OUT_DIM]),
                        op=mybir.AluOpType.mult,
                    )
                    if HAS_B1:
                        nc.vector.tensor_tensor(out=h[:], in0=h[:], in1=b1_rep[:],
                                                op=mybir.AluOpType.add)
                    # elu(h)+1 = exp(min(h,0)) + relu(h)  (-1 folded into bfp)
                    hm = pb_s.tile([128, HD], F32, tag="hm")
                    nc.vector.tensor_scalar_min(hm[:], h[:], 0.0)
                    em = pb_s.tile([128, HD], F32, tag="em")
                    nc.scalar.activation(em[:], hm[:],
                                         mybir.ActivationFunctionType.Exp)
                    rl = pb_s.tile([128, HD], F32, tag="rl")
                    nc.scalar.activation(rl[:], h[:],
                                         mybir.ActivationFunctionType.Relu)
                    e = pb_s.tile([128, HD], F32, tag="e")
                    nc.vector.tensor_tensor(out=e[:], in0=em[:], in1=rl[:],
                                            op=mybir.AluOpType.add)
                    scr = pb_s.tile([128, HD], F32, tag="scr")
                    nc.vector.tensor_tensor(out=scr[:], in0=e[:], in1=wf_rep[:],
                                            op=mybir.AluOpType.mult)
                    nc.vector.tensor_reduce(
                        out=y_all[:, w : w + 1], in_=scr[:],
                        axis=mybir.AxisListType.X, op=mybir.AluOpType.add,
                    )

                pending = []  # [(ps, rank), ...] epilogues of the previous group
                for goff, grp in gspec:
                    FAg = sum(fa for _, fa, _ in grp)
                    Fg = FAg + sum(fb for _, _, fb in grp)
                    o8 = goff * 8

                    gath = pb_gath.tile([128, G_MAX, ROW], BF16, tag="gath")
                    for c0 in range(0, FAg, GCHUNK):
                        cn = min(GCHUNK, FAg - c0)
                        nc.gpsimd.dma_gather(
                            out_ap=gath[:, c0 : c0 + cn, :], in_ap=tblA_d[:, :],
                            idxs_ap=idx_sb[:, o8 + c0 * 8 : o8 + (c0 + cn) * 8],
                            num_idxs=cn * 128, num_idxs_reg=cn * 128,
                            elem_size=ROW, queue_num=0,
                        )
                    for c0 in range(FAg, Fg, GCHUNK):
                        cn = min(GCHUNK, Fg - c0)
                        nc.gpsimd.dma_gather(
                            out_ap=gath[:, c0 : c0 + cn, :],
                            in_ap=tblB_d[:, :],
                            idxs_ap=idx_sb[:, o8 + c0 * 8 : o8 + (c0 + cn) * 8],
                            num_idxs=cn * 128, num_idxs_reg=cn * 128,
                            elem_size=ROW, queue_num=0,
                        )

                    # per-window segments: [A1 | A2 | B1 | B2]; each segment's
                    # DVE chain runs as soon as its gathers land, and its
                    # matmuls overlap the next segment's chain.
                    P = pb_m.tile([128, G_MAX, 16], BF16, tag="P")
                    ev = pb_m.tile([128, G_MAX, 8], BF16, tag="ev")
                    g = pb_g.tile([128, G_MAX, HD + HEADS], BF16, tag="g")
                    pss = [pb_ps.tile([128, HD + HEADS], F32, tag=f"ps{q}",
                                      name=f"psq{q}")
                           for q in range(len(grp))]

                    def seg(lo, hi, w, ps, is_first, is_last):
                        n = hi - lo
                        if n == 0:
                            return
                        nc.vector.tensor_tensor(
                            out=P[:, lo:hi, :],
                            in0=gath[:, lo:hi, EA_OFF : EA_OFF + 16],
                            in1=aloc[:, w : w + 1, :].to_broadcast([128, n, 16]),
                            op=mybir.AluOpType.mult,
                        )
                        nc.vector.tensor_tensor(
                            out=ev[:, lo:hi, :], in0=P[:, lo:hi, 0:8],
                            in1=P[:, lo:hi, 8:16],
                            op=mybir.AluOpType.max,
                        )
                        for hh in range(HEADS):
                            nc.vector.tensor_tensor(
                                out=g[:, lo:hi, hh * OUT_DIM : (hh + 1) * OUT_DIM]
                                    .rearrange("p f (a b) -> p f a b", b=2),
                                in0=gath[:, lo:hi, hh * OUT_DIM : (hh + 1) * OUT_DIM]
                                    .rearrange("p f (a b) -> p f a b", b=2),
                                in1=ev[:, lo:hi, 2 * hh : 2 * hh + 2][:, :, None, :]
                                    .to_broadcast([128, n, OUT_DIM // 2, 2]),
                                op=mybir.AluOpType.mult,
                            )
                        nc.scalar.activation(
                            g[:, lo:hi, HD:],
                            ev[:, lo:hi, :].rearrange("p f (h b) -> p f h b",
                                                      h=HEADS)[:, :, :, 0],
                            mybir.ActivationFunctionType.Copy,
                        )
                        for f in range(lo, hi):
                            nc.tensor.matmul(
                                out=ps[:], lhsT=ident[:], rhs=g[:, f, :],
                                start=(is_first and f == lo),
                                stop=(is_last and f == hi - 1),
                            )

                    ao = 0
                    for q, (w, fa, fb) in enumerate(grp):
                        seg(ao, ao + fa, w, pss[q], True, fb == 0)
                        ao += fa
                    bo = FAg
                    for q, (w, fa, fb) in enumerate(grp):
                        seg(bo, bo + fb, w, pss[q], fa == 0, True)
                        bo += fb

                    # software pipeline: the previous group's epilogues are
                    # emitted after this group's matmuls so the DVE queue
                    # never stalls on PSUM accumulation
                    for ps_, w_ in pending:
                        emit_epilogue(ps_, w_)
                    pending = [(pss[q], grp[q][0]) for q in range(len(grp))]
                for ps_, w_ in pending:
                    emit_epilogue(ps_, w_)

                nc.vector.tensor_scalar(
                    out=y_all[:], in0=y_all[:], scalar1=bfp_rep[:, :1], scalar2=None,
                    op0=mybir.AluOpType.add,
                )
                nc.sync.dma_start(
                    out=y_d[:, :].rearrange("(k p) o -> p (k o)", p=128),
                    in_=y_all[:, :],
                )

    nc.compile()
    return nc


_CACHE = {}


def kernel(**inputs) -> np.ndarray:
    x = np.asarray(inputs["x"], dtype=np.float32)
    edge_index = np.asarray(inputs["edge_index"])
    W1 = np.asarray(inputs["W1"], dtype=np.float32)
    attn_src = np.asarray(inputs["attn_src"], dtype=np.float32)
    attn_dst = np.asarray(inputs["attn_dst"], dtype=np.float32)
    b1 = np.asarray(inputs["b1"], dtype=np.float32)
    Wf = np.asarray(inputs["Wf"], dtype=np.float32)
    bf = np.asarray(inputs["bf"], dtype=np.float32)

    # fold attention vectors through W1; duplicate-pairs layout with the 0.2x
    # scaled copies for the leaky-relu max trick
    A_src = np.zeros((HD, HEADS), dtype=np.float32)
    A_dst = np.zeros((HD, HEADS), dtype=np.float32)
    for hh in range(HEADS):
        A_src[hh * OUT_DIM : (hh + 1) * OUT_DIM, hh] = attn_src[hh]
        A_dst[hh * OUT_DIM : (hh + 1) * OUT_DIM, hh] = attn_dst[hh]
    WA_src = W1 @ A_src  # [128, 4]
    WA_dst = W1 @ A_dst

    def duppairs(WA):
        # [128,4] -> [128,16]: [a0,a0,a1,a1,a2,a2,a3,a3, .2a0,.2a0,...]
        d = np.repeat(WA, 2, axis=1)  # [128, 8]
        return np.concatenate([d, 0.2 * d], axis=1)

    w1c = np.concatenate([W1, duppairs(WA_src)], axis=1).astype(np.float32)
    w1d = duppairs(WA_dst).astype(np.float32)
    bfp = (bf.reshape(()) - Wf.sum()).reshape(1, 1).astype(np.float32)

    gspec, idx_wrapped, node_of = preprocess(edge_index)

    has_b1 = bool(np.any(b1 != 0.0))
    pkey = (gspec, has_b1)
    if pkey not in _CACHE:
        _CACHE[pkey] = build_program(gspec, HAS_B1=has_b1)
    nc = _CACHE[pkey]

    xpad = np.zeros((NODES_PAD, IN_DIM), dtype=np.float32)
    xpad[:N_NODES] = x
    xT = np.ascontiguousarray(xpad.T).astype(ml_dtypes.bfloat16)

    in_maps = []
    for c in range(N_CORES):
        nids = node_of[c].reshape(-1)  # [6272], -1 for fakes
        xmy = np.zeros((NPC_PAD, IN_DIM), dtype=np.float32)
        valid = nids >= 0
        xmy[valid] = x[nids[valid]]
        xmyT = np.ascontiguousarray(xmy.T).astype(ml_dtypes.bfloat16)
        in_maps.append(
            {
                "xT": xT,
                "xmyT": xmyT,
                "w1c": w1c,
                "w1d": w1d,
                "b1": b1.reshape(1, HD),
                "wf": Wf.reshape(HD)[None, :].astype(np.float32),
                "bfp": bfp,
                "idx": idx_wrapped[c],
            }
        )

    res = bass_utils.run_bass_kernel_spmd(nc, in_maps, core_ids=list(range(N_CORES)))
    y = np.zeros((N_NODES, 1), dtype=np.float32)
    for c in range(N_CORES):
        yc = res.results[c]["y"].reshape(-1)  # [6272]
        nids = node_of[c].reshape(-1)
        valid = nids >= 0
        y[nids[valid], 0] = yc[valid]
    return y


# revision 7
# speedup vs baseline: 1.0195x; 1.0018x over previous
"""GAT layer kernel for Trainium2, 8 NeuronCores — dst-major edge layout.

Math (exact reformulation of the reference):
  proj = x @ W1;  a = proj . attn_src (per head);  d = proj . attn_dst
  score_e = leaky_relu(a[src] + d[dst], 0.2)
  exp(leaky_relu(s)) = max(exp(s), exp(0.2 s))       [monotonicity of exp]
                     = max(ea[src]*ed[dst], ea2[src]*ed2[dst])
  with ea = exp(a), ea2 = exp(0.2 a), ed = exp(d), ed2 = exp(0.2 d).
  (The segment-max subtraction is a softmax no-op and is dropped.)
  U[n]     = sum_{dst=n} expv_e * proj[src_e];  denom[n] = sum expv_e
  y = (elu(U/denom + b1) + 1) @ Wf + (bf - sum(Wf))

Device strategy:
  Nodes are assigned to (core, window, partition) by host permutation with
  windows of 128 dst nodes of similar (degA, degB) so per-partition edge
  lists pad tightly.  Edge slot (p, f): partition p = dst, f along free dim.
  Phase A builds a DRAM node-row table [proj bf16(128) | dup-pairs of
  ea/ea2 bf16(16) | pad] (512B rows) in two tables split at node 31744 so
  gather indices fit int16; row 0 of each is an all-zero sentinel used for
  padding slots (expv = 0).
  Phase B processes windows in pairs (one big-F with one small-F rank)
  sharing a gather region [A1|A2|B1|B2] so gather calls merge across the
  pair: dma_gather rows by src, expv = max(ea*ed, ea2*ed2) with the
  per-window ed table resident in SBUF, g = proj * expv, then per-slot
  identity-weight matmuls accumulate [U | denom] into PSUM (the identity
  stays loaded in the PE array for the whole phase), then normalize, bias,
  elu, and the final 128->1 projection, software-pipelined one group back.
"""

import numpy as np
import ml_dtypes

import concourse.bacc as bacc
import concourse.tile as tile
import concourse.mybir as mybir
import concourse.bass_utils as bass_utils
from concourse.masks import make_identity

# ---------------- problem constants ----------------
N_NODES = 50000
IN_DIM = 128
HEADS = 4
OUT_DIM = 32
HD = HEADS * OUT_DIM  # 128
N_CORES = 8
N_WIN = 49                      # windows per core
WIN = 128                       # dst nodes per window (one per partition)
NPC_PAD = N_WIN * WIN           # 6272 node slots per core
N_WIN_TOT = N_CORES * N_WIN     # 392
NODES_PAD = N_WIN_TOT * WIN     # 50176
SPLIT = 31744                   # node-id split for the two tables (31 chunks)
CHUNK = 1024                    # phase-A nodes per chunk (49 chunks)
ROW = 256                       # table row, bf16 elems (512B)
EA_OFF = 128                    # row cols 128:144 hold dup-pairs ea|ea2
TBLA_ROWS = SPLIT + 2           # sentinel + nodes [0, 31744)
TBLB_ROWS = (NODES_PAD - SPLIT) + 2  # sentinel + nodes [31744, 50176)
GCHUNK = 8                      # gather tiles per call (1024-idx ucode limit)
SCRATCH = 16384                 # swdge ring bytes (default; HW-fixed)

F32 = mybir.dt.float32
BF16 = mybir.dt.bfloat16
I16 = mybir.dt.int16


def _wrap_idx(idx_flat: np.ndarray) -> np.ndarray:
    """[n] int16 -> [128, n//16] wrapped (idx i at [i%16, i//16]), replicated
    down all 128 partitions (8 gpsimd core groups x 16)."""
    n = idx_flat.shape[0]
    assert n % 16 == 0
    w = np.zeros((128, n // 16), dtype=np.int16)
    w[:16, :] = idx_flat.reshape(n // 16, 16).T
    for r in range(1, 8):
        w[16 * r : 16 * (r + 1), :] = w[:16, :]
    return w


def preprocess(edge_index: np.ndarray):
    """Structure-only preprocessing: node permutation, window profiles and
    per-core wrapped gather indices."""
    src0 = edge_index[0].astype(np.int64)
    dst0 = edge_index[1].astype(np.int64)
    loops = np.arange(N_NODES, dtype=np.int64)
    src = np.concatenate([src0, dst0, loops])
    dst = np.concatenate([dst0, src0, loops])

    isB = src >= SPLIT
    degA = np.bincount(dst[~isB], minlength=N_NODES)
    degB = np.bincount(dst[isB], minlength=N_NODES)

    # windows of 128 nodes with similar (degA, degB): tight per-partition pads
    order = np.lexsort((-degB, -degA))          # primary: degA desc
    wFA = np.zeros(N_WIN_TOT, np.int64)
    wFB = np.zeros(N_WIN_TOT, np.int64)
    dA_pad = np.concatenate([degA[order], np.zeros(NODES_PAD - N_NODES, np.int64)])
    dB_pad = np.concatenate([degB[order], np.zeros(NODES_PAD - N_NODES, np.int64)])
    wFA = dA_pad.reshape(N_WIN_TOT, WIN).max(1)
    wFB = dB_pad.reshape(N_WIN_TOT, WIN).max(1)

    # windows sorted by (FA, FB) desc; window j -> core j%8 at rank j//8.
    # Groups of 8 consecutive windows share a rank, so the per-rank common
    # profile (max over the group) should stay tight for FA and FB jointly:
    # FA-sort in blocks of 64, FB-cluster within each block, then a small
    # deterministic swap anneal.
    wo = np.lexsort((-wFB, -wFA))
    out = []
    for b0 in range(0, N_WIN_TOT, 64):
        blk = wo[b0 : b0 + 64]
        out.append(blk[np.argsort(-wFB[blk], kind="stable")])
    worder = np.concatenate(out)

    def _prof_cost(w):
        fa = wFA[w].reshape(N_WIN, 8).max(1)
        fb = wFB[w].reshape(N_WIN, 8).max(1)
        return (fa + fb).sum()

    rng = np.random.default_rng(0)
    c = _prof_cost(worder)
    for _ in range(40000):
        i, j = rng.integers(0, N_WIN_TOT, 2)
        if i // 8 == j // 8:
            continue
        worder[i], worder[j] = worder[j], worder[i]
        c2 = _prof_cost(worder)
        if c2 <= c:
            c = c2
        else:
            worder[i], worder[j] = worder[j], worder[i]
    # common profile per rank: max over the 8 cores sharing the rank
    FA_prof = np.zeros(N_WIN, np.int64)
    FB_prof = np.zeros(N_WIN, np.int64)
    for k in range(N_WIN):
        grp = worder[k * 8 : (k + 1) * 8]
        FA_prof[k] = wFA[grp].max()
        FB_prof[k] = wFB[grp].max()
    F_prof = FA_prof + FB_prof

    # node -> (core, rank k, partition p)
    # global window gw (in sorted-node order) sits at rank k=pos//8, core=pos%8
    gw_rank = np.zeros(N_WIN_TOT, np.int64)
    gw_core = np.zeros(N_WIN_TOT, np.int64)
    for j, gw in enumerate(worder):
        gw_core[gw] = j % 8
        gw_rank[gw] = j // 8

    pos = np.empty(N_NODES, np.int64)           # node -> sorted position
    pos[order] = np.arange(N_NODES)
    node_gw = pos // WIN
    node_p = pos % WIN
    node_core = gw_core[node_gw]
    node_rank = gw_rank[node_gw]

    # map (core, rank, p) -> node for aloc/xmyT and output unpermute
    node_of = np.full((N_CORES, N_WIN, WIN), -1, np.int64)
    node_of[node_core, node_rank, node_p] = np.arange(N_NODES)

    # per-edge slot assignment
    e_core = node_core[dst]
    e_rank = node_rank[dst]
    e_p = node_p[dst]
    e_isB = isB.astype(np.int64)
    # rank within (dst, class): stable sort by (dst, class)
    ek = (pos[dst] << 1) | e_isB
    es = np.argsort(ek, kind="stable")
    ek_s = ek[es]
    new_run = np.ones(len(ek_s), dtype=bool)
    new_run[1:] = ek_s[1:] != ek_s[:-1]
    run_start = np.where(new_run)[0]
    start_per = run_start[np.cumsum(new_run) - 1]
    rank_in = np.arange(len(ek_s)) - start_per
    e_f = np.empty(len(src), np.int64)
    e_f[es] = rank_in

    # ---- rank pairing: groups of 2 ranks (big F with small F) share one
    # gather region laid out [A1 | A2 | B1 | B2], merging gather calls ----
    rsort = np.argsort(-F_prof, kind="stable")
    groups = []
    for i in range(N_WIN // 2):
        groups.append((int(rsort[i]), int(rsort[N_WIN - 1 - i])))
    # smallest (solo) group first: fills the pipeline quickly after the
    # phase-A barrier; the remaining pairs big-to-small so the drain is short
    groups = [(int(rsort[N_WIN // 2]),)] + groups

    # per-rank: group base offset contributions
    abase = np.zeros(N_WIN, np.int64)   # f-offset of rank's A segment
    bbase = np.zeros(N_WIN, np.int64)   # f-offset of rank's B segment
    goff = 0
    gspec = []  # per group: (off, [(rank, FA, FB), ...])
    for grp in groups:
        FAg = sum(int(FA_prof[r]) for r in grp)
        ao = goff
        for r in grp:
            abase[r] = ao
            ao += int(FA_prof[r])
        bo = goff + FAg
        for r in grp:
            bbase[r] = bo
            bo += int(FB_prof[r])
        gspec.append((goff, tuple((int(r), int(FA_prof[r]), int(FB_prof[r]))
                                  for r in grp)))
        goff = bo
    F_tot = int(goff)

    e_f = np.where(e_isB == 1, bbase[e_rank] + e_f, abase[e_rank] + e_f)

    # idx value: A: src+1 in tblA ; B: (src-SPLIT)+1 in tblB ; sentinel 0
    e_idx = np.where(e_isB == 1, src - SPLIT + 1, src + 1).astype(np.int16)

    e_slot = e_f * WIN + e_p

    idx_blob = np.zeros((N_CORES, F_tot * WIN), dtype=np.int16)
    idx_blob[e_core, e_slot] = e_idx

    idx_wrapped = np.stack([_wrap_idx(idx_blob[c]) for c in range(N_CORES)])

    return (
        tuple(gspec),
        idx_wrapped,
        node_of,
    )


# ---------------- bass program ----------------
def build_program(gspec, HAS_B1=True):
    # gspec: per group (offset_tiles, ((rank, FA, FB), ...1-2 windows))
    G_MAX = max(sum(fa + fb for _, fa, fb in grp) for _, grp in gspec)
    last_off, last_grp = gspec[-1]
    F_tot = last_off + sum(fa + fb for _, fa, fb in last_grp)

    nc = bacc.Bacc("TRN2", target_bir_lowering=False, debug=False, num_devices=1,
                   num_swdge_queues=1, dynamic_dma_scratch_size=SCRATCH)

    xT_d = nc.dram_tensor("xT", (IN_DIM, NODES_PAD), BF16, kind="ExternalInput").ap()
    xmyT_d = nc.dram_tensor("xmyT", (IN_DIM, NPC_PAD), BF16, kind="ExternalInput").ap()
    w1c_d = nc.dram_tensor("w1c", (IN_DIM, HD + 16), F32, kind="ExternalInput").ap()
    w1d_d = nc.dram_tensor("w1d", (IN_DIM, 16), F32, kind="ExternalInput").ap()
    b1_d = nc.dram_tensor("b1", (1, HD), F32, kind="ExternalInput").ap()
    wf_d = nc.dram_tensor("wf", (1, HD), F32, kind="ExternalInput").ap()
    bfp_d = nc.dram_tensor("bfp", (1, 1), F32, kind="ExternalInput").ap()
    idx_d = nc.dram_tensor("idx", (128, F_tot * 8), I16, kind="ExternalInput").ap()
    y_d = nc.dram_tensor("y", (NPC_PAD, 1), F32, kind="ExternalOutput").ap()
    tblA_d = nc.dram_tensor("tblA", (TBLA_ROWS, ROW), BF16, kind="Internal").ap()
    tblB_d = nc.dram_tensor("tblB", (TBLB_ROWS, ROW), BF16, kind="Internal").ap()

    with tile.TileContext(nc) as tc:
        with tc.tile_pool(name="const", bufs=1) as consts:
            ident = consts.tile([128, 128], BF16)
            make_identity(nc, ident[:])
            w1cf = consts.tile([128, HD + 16], F32)
            nc.sync.dma_start(out=w1cf[:], in_=w1c_d[:, :])
            w1c = consts.tile([128, HD + 16], BF16)
            nc.vector.tensor_copy(out=w1c[:], in_=w1cf[:])
            w1df = consts.tile([128, 16], F32)
            nc.sync.dma_start(out=w1df[:], in_=w1d_d[:, :])
            w1d = consts.tile([128, 16], BF16)
            nc.vector.tensor_copy(out=w1d[:], in_=w1df[:])

            # zero sentinels (row 0 of each table)
            zrow = consts.tile([1, ROW], BF16)
            nc.vector.memset(zrow[:], 0.0)
            nc.sync.dma_start(out=tblA_d[0:1, :], in_=zrow[:])
            nc.sync.dma_start(out=tblB_d[0:1, :], in_=zrow[:])

            # replicated epilogue params via ones-matmul
            ones_row = consts.tile([1, 128], F32)
            nc.vector.memset(ones_row[:], 1.0)
            b1_t = consts.tile([1, HD], F32)
            nc.sync.dma_start(out=b1_t[:], in_=b1_d[:, :])
            wf_t = consts.tile([1, HD], F32)
            nc.sync.dma_start(out=wf_t[:], in_=wf_d[:, :])
            bfp_t = consts.tile([1, 1], F32)
            nc.sync.dma_start(out=bfp_t[:], in_=bfp_d[:, :])
            with tc.tile_pool(name="bc_ps", bufs=1, space="PSUM") as bc_ps:
                bput = bc_ps.tile([128, 2 * HD + 1], F32)
                nc.tensor.matmul(out=bput[:, :HD], lhsT=ones_row[:, :], rhs=b1_t[:, :],
                                 start=True, stop=False)
                nc.tensor.matmul(out=bput[:, HD : 2 * HD], lhsT=ones_row[:, :],
                                 rhs=wf_t[:, :], start=False, stop=False)
                nc.tensor.matmul(out=bput[:, 2 * HD :], lhsT=ones_row[:, :],
                                 rhs=bfp_t[:, :], start=False, stop=True)
                b1_rep = consts.tile([128, HD], F32)
                nc.vector.tensor_copy(out=b1_rep[:], in_=bput[:, :HD])
                wf_rep = consts.tile([128, HD], F32)
                nc.vector.tensor_copy(out=wf_rep[:], in_=bput[:, HD : 2 * HD])
                bfp_rep = consts.tile([128, 1], F32)
                nc.vector.tensor_copy(out=bfp_rep[:], in_=bput[:, 2 * HD :])

            aloc = consts.tile([128, N_WIN, 16], BF16)
            y_all = consts.tile([128, N_WIN], F32)

            # idx blob: prefetch the whole thing into SBUF once
            idx_sb = consts.tile([128, F_tot * 8], I16)
            nc.sync.dma_start(out=idx_sb[:], in_=idx_d[:, :])

            # ---- phase A': per-core dst exp table (SBUF-resident) ----
            with (
                tc.tile_pool(name="pd_x", bufs=2) as pd_x,
                tc.tile_pool(name="pd_ps", bufs=2, space="PSUM") as pd_ps,
            ):
                xmt = pd_x.tile([128, NPC_PAD], BF16, tag="xmt")
                nc.sync.dma_start(out=xmt[:], in_=xmyT_d[:, :])
                for k in range(N_WIN):
                    ps = pd_ps.tile([128, 16], F32, tag="psd")
                    nc.tensor.matmul(
                        out=ps[:], lhsT=xmt[:, k * 128 : (k + 1) * 128],
                        rhs=w1d[:, :], start=True, stop=True,
                    )
                    nc.scalar.activation(aloc[:, k, :], ps[:],
                                         mybir.ActivationFunctionType.Exp)

            # ---- phase A: node tables (chunks of 1024 nodes) ----
            # manual row buffers: pad cols initialized once, full-row stores
            NROW = 5
            rows = [consts.tile([128, 8, ROW], BF16, name=f"rowbuf{i}")
                    for i in range(NROW)]
            for r in rows:
                nc.vector.memset(r[:, :, EA_OFF + 16 :], 0.0)
            with (
                tc.tile_pool(name="pa_x", bufs=8) as pa_x,
                tc.tile_pool(name="pa_ps", bufs=8, space="PSUM") as pa_ps,
            ):
                for c in range(49):
                    xt = pa_x.tile([128, CHUNK], BF16, tag="xt")
                    nc.sync.dma_start(out=xt[:], in_=xT_d[:, c * CHUNK : (c + 1) * CHUNK])
                    row = rows[c % NROW]
                    for i in range(4):  # pairs of 128-node tiles
                        ps = pa_ps.tile([128, 2, HD + 16], F32, tag="ps")
                        for j in range(2):
                            t = 2 * i + j
                            nc.tensor.matmul(
                                out=ps[:, j, :], lhsT=xt[:, t * 128 : (t + 1) * 128],
                                rhs=w1c[:, :], start=True, stop=True,
                            )
                        if i < 3:
                            nc.vector.tensor_copy(
                                out=row[:, 2 * i : 2 * i + 2, :HD], in_=ps[:, :, :HD])
                        else:
                            nc.scalar.activation(
                                row[:, 2 * i : 2 * i + 2, :HD], ps[:, :, :HD],
                                mybir.ActivationFunctionType.Copy)
                        nc.scalar.activation(row[:, 2 * i : 2 * i + 2,
                                                 EA_OFF : EA_OFF + 16],
                                             ps[:, :, HD : HD + 16],
                                             mybir.ActivationFunctionType.Exp)
                    # tblB stores go out on the Act DMA queue so the first
                    # A-table gathers only wait on the SP queue's tblA stores
                    if c < 31:
                        dst_ap = tblA_d[1 + c * CHUNK : 1 + (c + 1) * CHUNK, :]
                        eng = nc.sync
                    else:
                        c2 = c - 31
                        dst_ap = tblB_d[1 + c2 * CHUNK : 1 + (c2 + 1) * CHUNK, :]
                        eng = nc.scalar
                    eng.dma_start(
                        out=dst_ap.rearrange("(t p) e -> p t e", p=128),
                        in_=row[:],
                    )

            # ---- phase B ----
            with (
                tc.tile_pool(name="pb_gath", bufs=2) as pb_gath,
                tc.tile_pool(name="pb_g", bufs=2) as pb_g,
                tc.tile_pool(name="pb_m", bufs=3) as pb_m,
                tc.tile_pool(name="pb_s", bufs=3) as pb_s,
                tc.tile_pool(name="pb_ps", bufs=4, space="PSUM") as pb_ps,
            ):
                def emit_epilogue(ps, w):
                    dn = pb_s.tile([128, HEADS], F32, tag="dn")
                    nc.vector.tensor_scalar_max(dn[:], ps[:, HD:], 1e-12)
                    rec = pb_s.tile([128, HEADS, 1], F32, tag="rec")
                    nc.vector.reciprocal(rec[:, :, 0], dn[:])
                    h = pb_s.tile([128, HD], F32, tag="h")
                    nc.vector.tensor_tensor(
                        out=h[:],
                        in0=ps[:, :HD],
                        in1=rec[:, :, :1].to_broadcast([128, HEADS, OUT_DIM]),
                        op=mybir.AluOpType.mult,
                    )
                    if HAS_B1:
                        nc.vector.tensor_tensor(out=h[:], in0=h[:], in1=b1_rep[:],
                                                op=mybir.AluOpType.add)
                    # elu(h)+1 = exp(min(h,0)) + relu(h)  (-1 folded into bfp)
                    hm = pb_s.tile([128, HD], F32, tag="hm")
                    nc.vector.tensor_scalar_min(hm[:], h[:], 0.0)
                    em = pb_s.tile([128, HD], F32, tag="em")
                    nc.scalar.activation(em[:], hm[:],
                                         mybir.ActivationFunctionType.Exp)
                    rl = pb_s.tile([128, HD], F32, tag="rl")
                    nc.scalar.activation(rl[:], h[:],
                                         mybir.ActivationFunctionType.Relu)
                    e = pb_s.tile([128, HD], F32, tag="e")
                    nc.vector.tensor_tensor(out=e[:], in0=em[:], in1=rl[:],
                                            op=mybir.AluOpType.add)
                    scr = pb_s.tile([128, HD], F32, tag="scr")
                    nc.vector.tensor_tensor(out=scr[:], in0=e[:], in1=wf_rep[:],
                                            op=mybir.AluOpType.mult)
                    nc.vector.tensor_reduce(
                        out=y_all[:, w : w + 1], in_=scr[:],
                        axis=mybir.AxisListType.X, op=mybir.AluOpType.add,
                    )

                pending = []  # [(ps, rank), ...] epilogues of the previous group
                for goff, grp in gspec:
                    FAg = sum(fa for _, fa, _ in grp)
                    Fg = FAg + sum(fb for _, _, fb in grp)
                    o8 = goff * 8

                    gath = pb_gath.tile([128, G_MAX, ROW], BF16, tag="gath")
                    for c0 in range(0, FAg, GCHUNK):
                        cn = min(GCHUNK, FAg - c0)
                        nc.gpsimd.dma_gather(
                            out_ap=gath[:, c0 : c0 + cn, :], in_ap=tblA_d[:, :],
                            idxs_ap=idx_sb[:, o8 + c0 * 8 : o8 + (c0 + cn) * 8],
                            num_idxs=cn * 128, num_idxs_reg=cn * 128,
                            elem_size=ROW, queue_num=0,
                        )
                    for c0 in range(FAg, Fg, GCHUNK):
                        cn = min(GCHUNK, Fg - c0)
                        nc.gpsimd.dma_gather(
                            out_ap=gath[:, c0 : c0 + cn, :],
                            in_ap=tblB_d[:, :],
                            idxs_ap=idx_sb[:, o8 + c0 * 8 : o8 + (c0 + cn) * 8],
                            num_idxs=cn * 128, num_idxs_reg=cn * 128,
                            elem_size=ROW, queue_num=0,
                        )

                    # per-window segments: [A1 | A2 | B1 | B2]; each segment's
                    # DVE chain runs as soon as its gathers land, and its
                    # matmuls overlap the next segment's chain.
                    P = pb_m.tile([128, G_MAX, 16], BF16, tag="P")
                    ev = pb_m.tile([128, G_MAX, 8], BF16, tag="ev")
                    g = pb_g.tile([128, G_MAX, HD + HEADS], BF16, tag="g")
                    pss = [pb_ps.tile([128, HD + HEADS], F32, tag=f"ps{q}",
                                      name=f"psq{q}")
                           for q in range(len(grp))]

                    def seg(lo, hi, w, ps, is_first, is_last):
                        n = hi - lo
                        if n == 0:
                            return
                        nc.vector.tensor_tensor(
                            out=P[:, lo:hi, :],
                            in0=gath[:, lo:hi, EA_OFF : EA_OFF + 16],
                            in1=aloc[:, w : w + 1, :].to_broadcast([128, n, 16]),
                            op=mybir.AluOpType.mult,
                        )
                        nc.vector.tensor_tensor(
                            out=ev[:, lo:hi, :], in0=P[:, lo:hi, 0:8],
                            in1=P[:, lo:hi, 8:16],
                            op=mybir.AluOpType.max,
                        )
                        for hh in range(HEADS):
                            nc.vector.tensor_tensor(
                                out=g[:, lo:hi, hh * OUT_DIM : (hh + 1) * OUT_DIM]
                                    .rearrange("p f (a b) -> p f a b", b=2),
                                in0=gath[:, lo:hi, hh * OUT_DIM : (hh + 1) * OUT_DIM]
                                    .rearrange("p f (a b) -> p f a b", b=2),
                                in1=ev[:, lo:hi, 2 * hh : 2 * hh + 2][:, :, None, :]
                                    .to_broadcast([128, n, OUT_DIM // 2, 2]),
                                op=mybir.AluOpType.mult,
                            )
                        nc.scalar.activation(
                            g[:, lo:hi, HD:],
                            ev[:, lo:hi, :].rearrange("p f (h b) -> p f h b",
                                                      h=HEADS)[:, :, :, 0],
                            mybir.ActivationFunctionType.Copy,
                        )
                        for f in range(lo, hi):
                            nc.tensor.matmul(
                                out=ps[:], lhsT=ident[:], rhs=g[:, f, :],
                                start=(is_first and f == lo),
                                stop=(is_last and f == hi - 1),
                            )

                    ao = 0
                    for q, (w, fa, fb) in enumerate(grp):
                        seg(ao, ao + fa, w, pss[q], True, fb == 0)
                        ao += fa
                    bo = FAg
                    for q, (w, fa, fb) in enumerate(grp):
                        seg(bo, bo + fb, w, pss[q], fa == 0, True)
                        bo += fb

                    # software pipeline: the previous group's epilogues are
                    # emitted after this group's matmuls so the DVE queue
                    # never stalls on PSUM accumulation
                    for ps_, w_ in pending:
                        emit_epilogue(ps_, w_)
                    pending = [(pss[q], grp[q][0]) for q in range(len(grp))]
                for ps_, w_ in pending:
                    emit_epilogue(ps_, w_)

                nc.vector.tensor_scalar(
                    out=y_all[:], in0=y_all[:], scalar1=bfp_rep[:, :1], scalar2=None,
                    op0=mybir.AluOpType.add,
                )
                nc.sync.dma_start(
                    out=y_d[:, :].rearrange("(k p) o -> p (k o)", p=128),
                    in_=y_all[:, :],
                )

    nc.compile()
    return nc


_CACHE = {}


def kernel(**inputs) -> np.ndarray:
    x = np.asarray(inputs["x"], dtype=np.float32)
    edge_index = np.asarray(inputs["edge_index"])
    W1 = np.asarray(inputs["W1"], dtype=np.float32)
    attn_src = np.asarray(inputs["attn_src"], dtype=np.float32)
    attn_dst = np.asarray(inputs["attn_dst"], dtype=np.float32)
    b1 = np.asarray(inputs["b1"], dtype=np.float32)
    Wf = np.asarray(inputs["Wf"], dtype=np.float32)
    bf = np.asarray(inputs["bf"], dtype=np.float32)

    # fold attention vectors through W1; duplicate-pairs layout with the 0.2x
    # scaled copies for the leaky-relu max trick
    A_src = np.zeros((HD, HEADS), dtype=np.float32)
    A_dst = np.zeros((HD, HEADS), dtype=np.float32)
    for hh in range(HEADS):
        A_src[hh * OUT_DIM : (hh + 1) * OUT_DIM, hh] = attn_src[hh]
        A_dst[hh * OUT_DIM : (hh + 1) * OUT_DIM, hh] = attn_dst[hh]
    WA_src = W1 @ A_src  # [128, 4]
    WA_dst = W1 @ A_dst

    def duppairs(WA):
        # [128,4] -> [128,16]: [a0,a0,a1,a1,a2,a2,a3,a3, .2a0,.2a0,...]
        d = np.repeat(WA, 2, axis=1)  # [128, 8]
        return np.concatenate([d, 0.2 * d], axis=1)

    w1c = np.concatenate([W1, duppairs(WA_src)], axis=1).astype(np.float32)
    w1d = duppairs(WA_dst).astype(np.float32)
    bfp = (bf.reshape(()) - Wf.sum()).reshape(1, 1).astype(np.float32)

    gspec, idx_wrapped, node_of = preprocess(edge_index)

    has_b1 = bool(np.any(b1 != 0.0))
    pkey = (gspec, has_b1)
    if pkey not in _CACHE:
        _CACHE[pkey] = build_program(gspec, HAS_B1=has_b1)
    nc = _CACHE[pkey]

    xpad = np.zeros((NODES_PAD, IN_DIM), dtype=np.float32)
    xpad[:N_NODES] = x
    xT = np.ascontiguousarray(xpad.T).astype(ml_dtypes.bfloat16)

    in_maps = []
    for c in range(N_CORES):
        nids = node_of[c].reshape(-1)  # [6272], -1 for fakes
        xmy = np.zeros((NPC_PAD, IN_DIM), dtype=np.float32)
        valid = nids >= 0
        xmy[valid] = x[nids[valid]]
        xmyT = np.ascontiguousarray(xmy.T).astype(ml_dtypes.bfloat16)
        in_maps.append(
            {
                "xT": xT,
                "xmyT": xmyT,
                "w1c": w1c,
                "w1d": w1d,
                "b1": b1.reshape(1, HD),
                "wf": Wf.reshape(HD)[None, :].astype(np.float32),
                "bfp": bfp,
                "idx": idx_wrapped[c],
            }
        )

    res = bass_utils.run_bass_kernel_spmd(nc, in_maps, core_ids=list(range(N_CORES)))
    y = np.zeros((N_NODES, 1), dtype=np.float32)
    for c in range(N_CORES):
        yc = res.results[c]["y"].reshape(-1)  # [6272]
        nids = node_of[c].reshape(-1)
        valid = nids >= 0
        y[nids[valid], 0] = yc[valid]
    return y
